# revision 24
# baseline (speedup 1.0000x reference)
"""Trainium2 Bass kernel for nn_AcronymExpander (topk_masking).

Data-parallel over batch: 8 NeuronCores x 8 batch elements each; embedding
tables replicated. All gathers via gpsimd indirect DMA; all direct DMA on
HWDGE (sync). Per-core pipeline:
  A) BSG encoder: ctx gather -> PE-transpose -> matmul -> relu -> mean pool
     -> (mu, log sigma) per batch element.
  B) KL attention over G=1024 global tokens: gather [mu|logsig|1] rows,
     quad via DVE sub + ACT square-accumulate, KL in [128,8] tiles,
     stable softmax via exact min (PE transpose trick), top-8 via DVE max8,
     attention-weighted posterior via PE matmul with an ones-column in the
     gathered rows providing the softmax denominator.
  C) Long-form scoring: gather LF rows, L-pooling via constant block-diagonal
     matmuls, KL against the posterior, positional masking to -inf.
"""
import sys
import types

import numpy as np

# ---- shim: antenv.axon_hooks is absent on this image; bass_utils imports it
# when tracing is requested (BASS_TRACE=1 or trace=True). Provide it so
# profiling works instead of crashing.
if "antenv.axon_hooks" not in sys.modules:
    _hook_mod = types.ModuleType("antenv.axon_hooks")
    _hook_state = {"h": None}
    _hook_mod.set_axon_ntff_profile_hook = lambda h: _hook_state.__setitem__("h", h)
    _hook_mod.get_axon_ntff_profile_hook = lambda: _hook_state["h"]
    sys.modules["antenv.axon_hooks"] = _hook_mod
    try:
        from trn_agent_boot.trn_boot import _ntff_profile_via_ctypes
        _hook_mod.set_axon_ntff_profile_hook(
            _ntff_profile_via_ctypes("/opt/axon/libaxon_pjrt.so"))
    except Exception:
        pass

B, C, O, L, G = 64, 128, 64, 8, 1024
V, D, HID = 50000, 256, 256
NCORES, BS = 8, 8           # batch shard per core
K_TOP = 5
MASK_FILL = 1e5
PR = 264                    # prior-table row: mu(256) | logsig | 1.0 | pad(6)

_cache = {"nc": None}


def split_multi_waits(nc, mybir):
    """This walrus rejects >1 sync wait per instruction: move extras to NoOps."""
    for f in nc.m.functions:
        for blk in f.blocks:
            out, changed = [], False
            for inst in blk.instructions:
                si = inst.sync_info
                if si is not None and si.on_wait is not None and len(si.on_wait) > 1:
                    waits = list(si.on_wait)
                    for j, w in enumerate(waits[:-1]):
                        n = mybir.InstNoOp(name=f"{inst.name}-w{j}", ins=[], outs=[])
                        n.engine = inst.engine
                        n.sync_info = mybir.SyncInfo(on_wait=[w], on_update=[])
                        out.append(n)
                    inst.sync_info = mybir.SyncInfo(
                        on_wait=[waits[-1]], on_update=list(si.on_update or []))
                    changed = True
                out.append(inst)
            if changed:
                blk.instructions = out


def build_nc(split=True):
    import concourse.bass as bass
    import concourse.mybir as mybir
    from concourse.tile import TileContext

    f32 = mybir.dt.float32
    i32 = mybir.dt.int32
    ALU = mybir.AluOpType
    ACT = mybir.ActivationFunctionType

    nc = bass.Bass(dynamic_dma_scratch_size=131072)
    P = lambda n, s, dt=f32: nc.declare_dram_parameter(n, list(s), dt, isOutput=False)
    ptab = P("ptab", (V, PR))
    etab = P("etab", (V, D))
    fW = P("fW", (2 * D, HID))
    fb = P("fb", (1, HID))
    uvW = P("uvW", (HID, D + 1))
    uvb = P("uvb", (1, D + 1))
    ident = P("ident", (128, 128))
    onesq = P("onesq", (128, 128))
    pool4 = P("pool4", (512, 64))
    iota_b = P("iota_b", (BS, G))
    giota = P("giota", (128, 8))
    gidx = P("gidx", (128, BS * 8), i32)
    lfidx = P("lfidx", (128, BS * 4), i32)
    cidx = P("cidx", (128, BS), i32)
    sfidx = P("sfidx", (BS, 1), i32)
    gct = P("gct", (BS, 1))
    nout = P("nout", (BS, 1))
    lfctT = P("lfctT", (O, BS))
    score_out = nc.declare_dram_parameter("score_out", [BS, O], f32, isOutput=True)
    topw_out = nc.declare_dram_parameter("topw_out", [BS, K_TOP], mybir.dt.int32, isOutput=True)
    ed_all = nc.dram_tensor("ed_all", [BS, 128, 8], f32)
    dbsrc = nc.dram_tensor("dbsrc", [BS, D + 3], f32)
    dpost = nc.dram_tensor("dpost", [BS, D + 2], f32)

    with TileContext(nc) as tc:
        with (
            tc.tile_pool(name="sb", bufs=1) as sb,
            tc.tile_pool(name="ps", space="PSUM", bufs=1) as ps,
        ):
            # ---- persistent constants / weights ----
            def load(name, shape, src, dt=f32):
                t = sb.tile(list(shape), dt, tag=name)
                nc.sync.dma_start(t[:], src)
                return t

            fWt = [load(f"fw{k}", (128, HID), fW[k * 128:(k + 1) * 128, :]) for k in range(4)]
            uvt = [load(f"uv{k}", (128, D + 1), uvW[k * 128:(k + 1) * 128, :]) for k in range(2)]
            fb_t = load("fb", (1, HID), fb[:, :])
            uvb_t = load("uvb", (1, D + 1), uvb[:, :])
            id_t = load("id", (128, 128), ident[:, :])
            on_t = load("on", (128, 128), onesq[:, :])
            p4_t = [load(f"p4{t}", (128, 64), pool4[t * 128:(t + 1) * 128, :]) for t in range(4)]
            io_t = load("io", (BS, G), iota_b[:, :])
            gio_t = load("gio", (128, 8), giota[:, :])
            gidx_t = load("gidx", (128, BS * 8), gidx[:, :], i32)
            lfidx_t = load("lfidx", (128, BS * 4), lfidx[:, :], i32)
            cidx_t = load("cidx", (128, BS), cidx[:, :], i32)
            sfidx_t = load("sfidx", (BS, 1), sfidx[:, :], i32)
            gct_t = load("gct", (BS, 1), gct[:, :])
            nout_t = load("nout", (BS, 1), nout[:, :])
            lfct_t = load("lfct", (O, BS), lfctT[:, :])
            fill_t = sb.tile([128, 8], f32, tag="fill")
            nc.vector.memset(fill_t[:], MASK_FILL)
            ninf_t = sb.tile([BS, O], f32, tag="ninf")
            nc.vector.memset(ninf_t[:], float("-inf"))

            def gather(table, idx_col, rows, width, tag, bufs):
                t = sb.tile([rows, width], f32, tag=tag, bufs=bufs)
                nc.gpsimd.indirect_dma_start(
                    out=t[:], out_offset=None, in_=table[:, :],
                    in_offset=bass.IndirectOffsetOnAxis(ap=idx_col, axis=0))
                return t

            # ---- gathers for encoder ----
            cen = gather(etab, sfidx_t[:, 0:1], BS, D, "cen", 1)
            cxs = [gather(etab, cidx_t[:, b:b + 1], 128, D, "cx", 3) for b in range(BS)]

            # ---- stage A: encoder ----
            cT = []
            for k in range(2):
                tps = ps.tile([128, BS], f32, tag="psml", bufs=4)
                nc.tensor.transpose(tps[:], cen[:, k * 128:(k + 1) * 128], id_t[0:BS, 0:BS])
                t = sb.tile([128, BS], f32, tag="cT", bufs=2)
                nc.vector.tensor_copy(t[:], tps[:])
                cT.append(t)
            cw_ps = ps.tile([BS, HID], f32, tag="psml", bufs=4)
            nc.tensor.matmul(cw_ps[:], lhsT=cT[0][:], rhs=fWt[0][:], start=True, stop=False)
            nc.tensor.matmul(cw_ps[:], lhsT=cT[1][:], rhs=fWt[1][:], start=False, stop=False)
            nc.tensor.matmul(cw_ps[:], lhsT=on_t[0:1, 0:BS], rhs=fb_t[:, :], start=False, stop=True)
            cwb = sb.tile([BS, HID], f32, tag="cwb")
            nc.vector.tensor_copy(cwb[:], cw_ps[:])
            cwrows = []
            for b in range(BS):
                r = sb.tile([1, HID], f32, tag="cwr", bufs=BS, name=f"cwr{b}")
                nc.sync.dma_start(r[:], cwb[b:b + 1, :])
                cwrows.append(r)

            plT_ps = [ps.tile([128, BS], f32, tag="plt", bufs=2, name=f"plTps{k}") for k in range(2)]
            for b in range(BS):
                cxT = []
                for k in range(2):
                    tps = ps.tile([128, 128], f32, tag="pbig", bufs=2)
                    nc.tensor.transpose(tps[:], cxs[b][:, k * 128:(k + 1) * 128], id_t[:, :])
                    t = sb.tile([128, 128], f32, tag="cxT", bufs=2)
                    nc.vector.tensor_copy(t[:], tps[:])
                    cxT.append(t)
                h_ps = ps.tile([128, HID], f32, tag="pbig", bufs=2)
                nc.tensor.matmul(h_ps[:], lhsT=cxT[0][:], rhs=fWt[2][:], start=True, stop=False)
                nc.tensor.matmul(h_ps[:], lhsT=cxT[1][:], rhs=fWt[3][:], start=False, stop=False)
                nc.tensor.matmul(h_ps[:], lhsT=on_t[0:1, :], rhs=cwrows[b][:], start=False, stop=True)
                hr = sb.tile([128, HID], f32, tag="hr", bufs=2)
                nc.scalar.activation(hr[:], h_ps[:], ACT.Relu)
                for k in range(2):
                    nc.tensor.matmul(plT_ps[k][:, b:b + 1],
                                     lhsT=hr[:, k * 128:(k + 1) * 128],
                                     rhs=on_t[:, 0:1], start=True, stop=True)
            plT = []
            for k in range(2):
                t = sb.tile([128, BS], f32, tag="plT", bufs=2)
                nc.scalar.mul(t[:], plT_ps[k][:], 1.0 / C)
                plT.append(t)
            uv_ps = ps.tile([BS, D + 1], f32, tag="psml", bufs=4)
            nc.tensor.matmul(uv_ps[:], lhsT=plT[0][:], rhs=uvt[0][:], start=True, stop=False)
            nc.tensor.matmul(uv_ps[:], lhsT=plT[1][:], rhs=uvt[1][:], start=False, stop=False)
            nc.tensor.matmul(uv_ps[:], lhsT=on_t[0:1, 0:BS], rhs=uvb_t[:, :], start=False, stop=True)

            psig = sb.tile([BS, 1], f32, tag="psig")
            nc.vector.tensor_copy(psig[:], uv_ps[:, D:D + 1])
            sqq = sb.tile([BS, 1], f32, tag="sqq")
            nc.scalar.activation(sqq[:], psig[:], ACT.Exp, scale=2.0)
            c1 = sb.tile([BS, 1], f32, tag="c1")
            nc.vector.tensor_scalar(c1[:], psig[:], -256.0, -128.0, op0=ALU.mult, op1=ALU.add)
            c2 = sb.tile([BS, 1], f32, tag="c2")
            nc.vector.tensor_scalar(c2[:], sqq[:], 128.0, None, op0=ALU.mult)
            bsrc = sb.tile([BS, D + 3], f32, tag="bsrc")
            nc.vector.tensor_copy(bsrc[:, 0:D], uv_ps[:, 0:D])
            nc.vector.tensor_copy(bsrc[:, D:D + 1], c1[:])
            nc.vector.tensor_copy(bsrc[:, D + 1:D + 2], c2[:])
            nc.vector.tensor_copy(bsrc[:, D + 2:D + 3], gct_t[:])

            nc.sync.dma_start(dbsrc[:, :], bsrc[:])
            score_cols = sb.tile([O, BS], f32, tag="scol")

            # ---- stages B & C per batch element ----
            SR = PR  # 264: gathered row stride inside the per-b big tiles
            nrm_all = sb.tile([O, BS], f32, tag="nrmall")
            nc.vector.tensor_scalar(nrm_all[:], lfct_t[:], 1.0, None, op0=ALU.max)
            rcn_all = sb.tile([O, BS], f32, tag="rcnall")
            nc.vector.reciprocal(rcn_all[:], nrm_all[:])
            qo_all = sb.tile([O, BS], f32, tag="qoall")
            sgn_all = sb.tile([O, BS], f32, tag="sgnall")

            def sview(ap, off, dims):
                return bass.AP(ap.tensor, ap.offset + off, [list(ap.ap[0])] + dims)

            for b in range(BS):
                bc_sb = sb.tile([128, D + 3], f32, tag="bcs", bufs=3)
                nc.sync.dma_start(
                    bc_sb[:], bass.AP(dbsrc[:, :].tensor, b * (D + 3), [[0, 128], [1, D + 3]]))
                gbig = sb.tile([128, 8 * SR], f32, tag="gb", bufs=3)
                for t in range(8):
                    nc.gpsimd.indirect_dma_start(
                        out=gbig[:, t * SR:(t + 1) * SR], out_offset=None, in_=ptab[:, :],
                        in_offset=bass.IndirectOffsetOnAxis(
                            ap=gidx_t[:, 8 * b + t:8 * b + t + 1], axis=0))
                lsgv = sview(gbig[:], D, [[SR, 8]])               # [128, 8] logsig view
                gmuv = sview(gbig[:], 0, [[SR, 8], [1, D]])       # [128, 8, 256]
                bcv = sview(bc_sb[:], 0, [[0, 8], [1, D]])        # bcast along tile dim
                dscb = sb.tile([128, 8, D], f32, tag="dscb", bufs=1)
                nc.vector.tensor_tensor(out=dscb[:], in0=gmuv, in1=bcv, op=ALU.subtract)
                quad = sb.tile([128, 8], f32, tag="quad", bufs=2)
                for t in range(8):
                    dsq = sb.tile([128, D], f32, tag="dsq", bufs=1)
                    nc.scalar.activation(dsq[:], dscb[:, t, :], ACT.Square,
                                         accum_out=quad[:, t:t + 1])
                dm1 = ps.tile([1, 1], f32, tag="psml", bufs=4, name=f"dm1_{b}")
                nc.tensor.matmul(dm1[:], lhsT=quad[:, 0:1], rhs=on_t[:, 0:1], start=True, stop=True)
                # kl on [128, 8]
                einv = sb.tile([128, 8], f32, tag="einv", bufs=2)
                nc.scalar.activation(einv[:], lsgv, ACT.Exp, scale=-2.0)
                t1 = sb.tile([128, 8], f32, tag="t1", bufs=2)
                nc.vector.tensor_scalar(t1[:], quad[:], 0.5, bc_sb[:, D + 1:D + 2],
                                        op0=ALU.mult, op1=ALU.add)
                t2 = sb.tile([128, 8], f32, tag="t2", bufs=2)
                nc.vector.tensor_mul(t2[:], t1[:], einv[:])
                t3 = sb.tile([128, 8], f32, tag="t3", bufs=2)
                nc.vector.tensor_scalar(t3[:], lsgv, 256.0, bc_sb[:, D:D + 1],
                                        op0=ALU.mult, op1=ALU.add)
                klb = sb.tile([128, 8], f32, tag="klb", bufs=2)
                nc.vector.tensor_add(klb[:], t2[:], t3[:])
                cmpm = sb.tile([128, 8], mybir.dt.uint32, tag="cmpm", bufs=2)
                nc.vector.tensor_tensor(
                    out=cmpm[:], in0=gio_t[:], in1=bc_sb[:, D + 2:D + 3].to_broadcast([128, 8]),
                    op=ALU.is_ge)
                nc.vector.copy_predicated(klb[:], cmpm[:], fill_t[:])
                # exact min over 1024
                mn1 = sb.tile([128, 1], f32, tag="mn1", bufs=2)
                nc.vector.tensor_reduce(mn1[:], klb[:], axis=mybir.AxisListType.X, op=ALU.min)
                mn_ps = ps.tile([1, 128], f32, tag="psml", bufs=4)
                nc.tensor.transpose(mn_ps[:], mn1[:], id_t[:, :])
                mns = sb.tile([1, 1], f32, tag="mns", bufs=2)
                nc.vector.tensor_reduce(mns[:], mn_ps[:], axis=mybir.AxisListType.X, op=ALU.min)
                # Match XLA/Eigen exp semantics: gradual subnormals, hard 0 at
                # x <= -97.2865 (measured cutoff, identical on cpu + neuron).
                # ACT's exp spline clamps before f32 underflow, so compute
                # exp(x/2)^2 and apply the cutoff mask explicitly.
                mrow = sb.tile([1, 2], f32, tag="mrow", bufs=2)
                nc.vector.tensor_scalar(mrow[:, 0:1], mns[:], 0.5, None, op0=ALU.mult)
                nc.vector.tensor_scalar(mrow[:, 1:2], mns[:], 1.0, 97.2865, op0=ALU.mult, op1=ALU.add)
                mnb_ps = ps.tile([128, 2], f32, tag="psml", bufs=4)
                nc.tensor.matmul(mnb_ps[:], lhsT=on_t[0:1, :], rhs=mrow[:], start=True, stop=True)
                mnb = sb.tile([128, 2], f32, tag="mnb", bufs=2)
                nc.vector.tensor_copy(mnb[:], mnb_ps[:])
                e_h = sb.tile([128, 8], f32, tag="eh", bufs=2)
                nc.scalar.activation(e_h[:], klb[:], ACT.Exp, scale=-0.5, bias=mnb[:, 0:1])
                e_sq = sb.tile([128, 8], f32, tag="esq", bufs=2)
                nc.vector.tensor_mul(e_sq[:], e_h[:], e_h[:])
                ecut = sb.tile([128, 8], f32, tag="ecut", bufs=2)
                nc.vector.tensor_scalar(ecut[:], klb[:], mnb[:, 1:2], None, op0=ALU.is_lt)
                e_b = sb.tile([128, 8], f32, tag="eb", bufs=2)
                nc.vector.tensor_mul(e_b[:], e_sq[:], ecut[:])
                # weighted posterior accumulate: [sum e*mu | sum e*ls | sum e]
                ws_ps = ps.tile([1, D + 2], f32, tag="psml", bufs=4)
                for t in range(8):
                    nc.tensor.matmul(ws_ps[:], lhsT=e_b[:, t:t + 1],
                                     rhs=gbig[:, t * SR:t * SR + D + 2],
                                     start=(t == 0), stop=(t == 7))
                dm2 = ps.tile([1, 1], f32, tag="psml", bufs=4, name=f"dm2_{b}")
                nc.tensor.matmul(dm2[:], lhsT=e_b[:, 0:1], rhs=on_t[:, 0:1], start=True, stop=True)
                sigx = sb.tile([128, 8], f32, tag="sigx", bufs=2)
                nc.scalar.activation(sigx[:], lsgv, ACT.Exp)
                scr2 = sb.tile([128, 8], f32, tag="scr2", bufs=2)
                pcol = sb.tile([128, 1], f32, tag="pcol", bufs=2)
                nc.vector.tensor_mul(scr2[:], e_b[:], sigx[:])
                nc.vector.tensor_reduce(pcol[:], scr2[:], axis=mybir.AxisListType.X, op=ALU.add)
                sg_ps = ps.tile([1, 1], f32, tag="psml", bufs=4)
                nc.tensor.matmul(sg_ps[:], lhsT=pcol[:], rhs=on_t[:, 0:1], start=True, stop=True)
                rc = sb.tile([1, 1], f32, tag="rc", bufs=2)
                nc.vector.reciprocal(rc[:], ws_ps[:, D + 1:D + 2])
                # posterior row: mu_post(256) | cc1 | cc2
                post = sb.tile([1, D + 2], f32, tag="post", bufs=2)
                nc.vector.tensor_scalar(post[:, 0:D], ws_ps[:, 0:D], rc[:], None, op0=ALU.mult)
                sgp = sb.tile([1, 1], f32, tag="sgp", bufs=2)
                nc.vector.tensor_scalar(sgp[:], sg_ps[:], rc[:], None, op0=ALU.mult)
                lnsg = sb.tile([1, 1], f32, tag="lnsg", bufs=2)
                nc.scalar.activation(lnsg[:], sgp[:], ACT.Ln)
                nc.vector.tensor_scalar(post[:, D:D + 1], lnsg[:], -256.0, -128.0,
                                        op0=ALU.mult, op1=ALU.add)
                sq2b = sb.tile([1, 1], f32, tag="sq2b", bufs=2)
                nc.vector.tensor_mul(sq2b[:], sgp[:], sgp[:])
                nc.vector.tensor_scalar(post[:, D + 1:D + 2], sq2b[:], 128.0, None, op0=ALU.mult)
                nc.sync.dma_start(ed_all[b, :, :], e_b[:])

                # ---- stage C (per-b part) ----
                nc.sync.dma_start(dpost[b:b + 1, :], post[:])
                bc2_sb = sb.tile([O, D + 2], f32, tag="bc2s", bufs=3)
                nc.sync.dma_start(
                    bc2_sb[:], bass.AP(dpost[:, :].tensor, b * (D + 2), [[0, O], [1, D + 2]]))
                lbig = sb.tile([128, 4 * SR], f32, tag="lb", bufs=2)
                for t in range(4):
                    nc.gpsimd.indirect_dma_start(
                        out=lbig[:, t * SR:(t + 1) * SR], out_offset=None, in_=ptab[:, :],
                        in_offset=bass.IndirectOffsetOnAxis(
                            ap=lfidx_t[:, 4 * b + t:4 * b + t + 1], axis=0))
                nc.scalar.activation(sview(lbig[:], D + 2, [[SR, 4]]),
                                     sview(lbig[:], D, [[SR, 4]]), ACT.Exp)
                lf_ps = ps.tile([O, D + 3], f32, tag="pbig", bufs=2)
                for t in range(4):
                    nc.tensor.matmul(lf_ps[:], lhsT=p4_t[t][:], rhs=lbig[:, t * SR:t * SR + D + 3],
                                     start=(t == 0), stop=(t == 3))
                d1 = sb.tile([O, D + 3], f32, tag="d1", bufs=2)
                nc.vector.tensor_scalar(d1[:], lf_ps[:], rcn_all[:, b:b + 1], None, op0=ALU.mult)
                dm3 = ps.tile([1, 1], f32, tag="psml", bufs=4, name=f"dm3_{b}")
                nc.tensor.matmul(dm3[:], lhsT=d1[:, 0:1], rhs=on_t[0:O, 0:1], start=True, stop=True)
                d2 = sb.tile([O, D], f32, tag="d2", bufs=2)
                nc.vector.tensor_sub(d2[:], d1[:, 0:D], bc2_sb[:, 0:D])
                d2sq = sb.tile([O, D], f32, tag="d2sq", bufs=2)
                nc.scalar.activation(d2sq[:], d2[:], ACT.Square, accum_out=qo_all[:, b:b + 1])
                nc.vector.tensor_copy(sgn_all[:, b:b + 1], d1[:, D + 2:D + 3])

            # ---- stage C batched tail ----
            cc1_all = sb.tile([O, BS], f32, tag="cc1all")
            nc.sync.dma_start(cc1_all[:], bass.AP(dpost[:, :].tensor, D, [[0, O], [D + 2, BS]]))
            cc2_all = sb.tile([O, BS], f32, tag="cc2all")
            nc.sync.dma_start(cc2_all[:], bass.AP(dpost[:, :].tensor, D + 1, [[0, O], [D + 2, BS]]))
            lns_a = sb.tile([O, BS], f32, tag="lnsa")
            nc.scalar.activation(lns_a[:], sgn_all[:], ACT.Ln)
            sq2_a = sb.tile([O, BS], f32, tag="sq2a")
            nc.vector.tensor_mul(sq2_a[:], sgn_all[:], sgn_all[:])
            isq_a = sb.tile([O, BS], f32, tag="isqa")
            nc.vector.reciprocal(isq_a[:], sq2_a[:])
            u1_a = sb.tile([O, BS], f32, tag="u1a")
            nc.vector.tensor_scalar(u1_a[:], qo_all[:], 0.5, None, op0=ALU.mult)
            nc.vector.tensor_add(u1_a[:], u1_a[:], cc2_all[:])
            u2_a = sb.tile([O, BS], f32, tag="u2a")
            nc.vector.tensor_mul(u2_a[:], u1_a[:], isq_a[:])
            u3_a = sb.tile([O, BS], f32, tag="u3a")
            nc.vector.tensor_scalar(u3_a[:], lns_a[:], 256.0, None, op0=ALU.mult)
            nc.vector.tensor_add(u3_a[:], u3_a[:], cc1_all[:])
            klo_a = sb.tile([O, BS], f32, tag="kloa")
            nc.vector.tensor_add(klo_a[:], u2_a[:], u3_a[:])
            nc.vector.tensor_scalar(score_cols[:], klo_a[:], -1.0, None, op0=ALU.mult)

            # ---- batched top-5 indices ----
            erow_all = sb.tile([BS, G], f32, tag="erowall")
            for b in range(BS):
                nc.sync.dma_start(erow_all[b:b + 1, :],
                                  ed_all[b, :, :].rearrange("p t -> t p"))
            t8 = sb.tile([BS, 8], f32, tag="t8")
            nc.vector.max(t8[:], erow_all[:])
            t8i = sb.tile([BS, 8], mybir.dt.uint32, tag="t8i")
            nc.vector.max_index(t8i[:], t8[:], erow_all[:])
            tw = sb.tile([BS, K_TOP], mybir.dt.int32, tag="tw")
            nc.vector.tensor_copy(tw[:], t8i[:, 0:K_TOP])
            nc.sync.dma_start(topw_out[:, :], tw[:])

            # ---- finalize score ----
            sc_ps = ps.tile([BS, O], f32, tag="psml", bufs=4)
            nc.tensor.transpose(sc_ps[:], score_cols[:], id_t[0:O, 0:O])
            scs = sb.tile([BS, O], f32, tag="scs")
            nc.vector.tensor_copy(scs[:], sc_ps[:])
            cmp2 = sb.tile([BS, O], mybir.dt.uint32, tag="cmp2")
            nc.vector.tensor_scalar(cmp2[:], io_t[:, 0:O], nout_t[:], None, op0=ALU.is_ge)
            nc.vector.copy_predicated(scs[:], cmp2[:], ninf_t[:])
            nc.sync.dma_start(score_out[:, :], scs[:])

    if split:
        split_multi_waits(nc, mybir)
    return nc


def host_prep(inputs):
    """Build per-core in_maps (layout/dtype transforms only)."""
    f = lambda x: np.ascontiguousarray(np.asarray(x), dtype=np.float32)
    i = lambda x: np.ascontiguousarray(np.asarray(x), dtype=np.int32)

    ptab = np.zeros((V, PR), np.float32)
    ptab[:, :D] = np.asarray(inputs["emb_mu"], np.float32)
    ptab[:, D] = np.asarray(inputs["emb_log_sigma"], np.float32)[:, 0]
    ptab[:, D + 1] = 1.0
    etab = f(inputs["enc_emb"])
    fW = f(inputs["f_W"])
    fb = f(inputs["f_b"]).reshape(1, HID)
    uvW = np.concatenate([f(inputs["u_W"]), f(inputs["v_W"])], axis=1)
    uvb = np.concatenate([f(inputs["u_b"]), f(inputs["v_b"])]).reshape(1, D + 1)
    ident = np.eye(128, dtype=np.float32)
    onesq = np.ones((128, 128), np.float32)
    pool4 = (np.arange(512)[:, None] // L == np.arange(O)[None, :]).astype(np.float32)
    iota_b = np.tile(np.arange(G, dtype=np.float32), (BS, 1))
    giota = (np.arange(8)[None, :] * 128 + np.arange(128)[:, None]).astype(np.float32)

    sf_ids = i(inputs["sf_ids"])
    context_ids = i(inputs["context_ids"])
    lf_ids = i(inputs["lf_ids"])
    global_ids = i(inputs["global_ids"])
    lf_token_ct = f(inputs["lf_token_ct"])
    gct_all = f(inputs["global_token_ct"]).reshape(B)
    nout_all = f(inputs["num_outputs"]).reshape(B)

    shared = dict(ptab=ptab, etab=etab, fW=fW, fb=fb, uvW=uvW, uvb=uvb,
                  ident=ident, onesq=onesq, pool4=pool4, iota_b=iota_b,
                  giota=giota)
    in_maps = []
    for c in range(NCORES):
        s = slice(c * BS, (c + 1) * BS)
        gi = global_ids[s].reshape(BS, 8, 128).transpose(2, 0, 1).reshape(128, BS * 8)
        li = lf_ids[s].reshape(BS, 512).reshape(BS, 4, 128).transpose(2, 0, 1).reshape(128, BS * 4)
        ci = context_ids[s].T
        in_maps.append(dict(
            shared,
            gidx=np.ascontiguousarray(gi), lfidx=np.ascontiguousarray(li),
            cidx=np.ascontiguousarray(ci),
            sfidx=np.ascontiguousarray(sf_ids[s].reshape(BS, 1)),
            gct=np.ascontiguousarray(gct_all[s].reshape(BS, 1)),
            nout=np.ascontiguousarray(nout_all[s].reshape(BS, 1)),
            lfctT=np.ascontiguousarray(lf_token_ct[s].T),
        ))
    return in_maps


def kernel(**inputs):
    from concourse.bass_utils import run_bass_kernel_spmd

    in_maps = host_prep(inputs)
    if _cache["nc"] is None:
        _cache["nc"] = build_nc()
    res = run_bass_kernel_spmd(_cache["nc"], in_maps, core_ids=list(range(NCORES)))
    _cache["last_res"] = res
    score = np.concatenate([r["score_out"] for r in res.results], axis=0)
    topw = np.concatenate([r["topw_out"] for r in res.results], axis=0)
    return score, np.asarray(inputs["target_lf_ids"]), topw


# revision 25
# speedup vs baseline: 1.1831x; 1.1831x over previous
"""Trainium2 Bass kernel for nn_AcronymExpander (topk_masking).

Data-parallel over batch: 8 NeuronCores x 8 batch elements each; embedding
tables replicated. All gathers via gpsimd indirect DMA; all direct DMA on
HWDGE (sync). Per-core pipeline:
  A) BSG encoder: ctx gather -> PE-transpose -> matmul -> relu -> mean pool
     -> (mu, log sigma) per batch element.
  B) KL attention over G=1024 global tokens: gather [mu|logsig|1] rows,
     quad via DVE sub + ACT square-accumulate, KL in [128,8] tiles,
     stable softmax via exact min (PE transpose trick), top-8 via DVE max8,
     attention-weighted posterior via PE matmul with an ones-column in the
     gathered rows providing the softmax denominator.
  C) Long-form scoring: gather LF rows, L-pooling via constant block-diagonal
     matmuls, KL against the posterior, positional masking to -inf.
"""
import sys
import types

import numpy as np

# ---- shim: antenv.axon_hooks is absent on this image; bass_utils imports it
# when tracing is requested (BASS_TRACE=1 or trace=True). Provide it so
# profiling works instead of crashing.
if "antenv.axon_hooks" not in sys.modules:
    _hook_mod = types.ModuleType("antenv.axon_hooks")
    _hook_state = {"h": None}
    _hook_mod.set_axon_ntff_profile_hook = lambda h: _hook_state.__setitem__("h", h)
    _hook_mod.get_axon_ntff_profile_hook = lambda: _hook_state["h"]
    sys.modules["antenv.axon_hooks"] = _hook_mod
    try:
        from trn_agent_boot.trn_boot import _ntff_profile_via_ctypes
        _hook_mod.set_axon_ntff_profile_hook(
            _ntff_profile_via_ctypes("/opt/axon/libaxon_pjrt.so"))
    except Exception:
        pass

B, C, O, L, G = 64, 128, 64, 8, 1024
V, D, HID = 50000, 256, 256
NCORES, BS = 8, 8           # batch shard per core
K_TOP = 5
MASK_FILL = 1e5
PR = 264                    # prior-table row: mu(256) | logsig | 1.0 | pad(6)

_cache = {"nc": None}


def split_multi_waits(nc, mybir):
    """This walrus rejects >1 sync wait per instruction: move extras to NoOps."""
    for f in nc.m.functions:
        for blk in f.blocks:
            out, changed = [], False
            for inst in blk.instructions:
                si = inst.sync_info
                if si is not None and si.on_wait is not None and len(si.on_wait) > 1:
                    waits = list(si.on_wait)
                    for j, w in enumerate(waits[:-1]):
                        n = mybir.InstNoOp(name=f"{inst.name}-w{j}", ins=[], outs=[])
                        n.engine = inst.engine
                        n.sync_info = mybir.SyncInfo(on_wait=[w], on_update=[])
                        out.append(n)
                    inst.sync_info = mybir.SyncInfo(
                        on_wait=[waits[-1]], on_update=list(si.on_update or []))
                    changed = True
                out.append(inst)
            if changed:
                blk.instructions = out


def build_nc(split=True):
    import concourse.bass as bass
    import concourse.mybir as mybir
    from concourse.tile import TileContext

    f32 = mybir.dt.float32
    i32 = mybir.dt.int32
    ALU = mybir.AluOpType
    ACT = mybir.ActivationFunctionType

    nc = bass.Bass(dynamic_dma_scratch_size=131072)
    P = lambda n, s, dt=f32: nc.declare_dram_parameter(n, list(s), dt, isOutput=False)
    ptab = P("ptab", (V, PR))
    etab = P("etab", (V, D))
    fW = P("fW", (2 * D, HID))
    fb = P("fb", (1, HID))
    uvW = P("uvW", (HID, D + 1))
    uvb = P("uvb", (1, D + 1))
    ident = P("ident", (128, 128))
    onesq = P("onesq", (128, 128))
    pool4 = P("pool4", (512, 64))
    iota_b = P("iota_b", (BS, G))
    giota = P("giota", (128, 8))
    gidx = P("gidx", (128, BS * 8), i32)
    lfidx = P("lfidx", (128, BS * 4), i32)
    cidx = P("cidx", (128, BS), i32)
    sfidx = P("sfidx", (BS, 1), i32)
    gct = P("gct", (BS, 1))
    nout = P("nout", (BS, 1))
    lfctT = P("lfctT", (O, BS))
    score_out = nc.declare_dram_parameter("score_out", [BS, O], f32, isOutput=True)
    topw_out = nc.declare_dram_parameter("topw_out", [BS, K_TOP], mybir.dt.int32, isOutput=True)
    ed_all = nc.dram_tensor("ed_all", [BS, 128, 8], f32)
    dbsrc = nc.dram_tensor("dbsrc", [BS, D + 3], f32)
    dpost = nc.dram_tensor("dpost", [BS, D + 2], f32)

    with TileContext(nc) as tc:
        with (
            tc.tile_pool(name="sb", bufs=1) as sb,
            tc.tile_pool(name="ps", space="PSUM", bufs=1) as ps,
        ):
            # ---- persistent constants / weights ----
            def load(name, shape, src, dt=f32):
                t = sb.tile(list(shape), dt, tag=name)
                nc.sync.dma_start(t[:], src)
                return t

            fWt = [load(f"fw{k}", (128, HID), fW[k * 128:(k + 1) * 128, :]) for k in range(4)]
            uvt = [load(f"uv{k}", (128, D + 1), uvW[k * 128:(k + 1) * 128, :]) for k in range(2)]
            fb_t = load("fb", (1, HID), fb[:, :])
            uvb_t = load("uvb", (1, D + 1), uvb[:, :])
            id_t = load("id", (128, 128), ident[:, :])
            on_t = load("on", (128, 128), onesq[:, :])
            p4_t = [load(f"p4{t}", (128, 64), pool4[t * 128:(t + 1) * 128, :]) for t in range(4)]
            io_t = load("io", (BS, G), iota_b[:, :])
            gio_t = load("gio", (128, 8), giota[:, :])
            gidx_t = load("gidx", (128, BS * 8), gidx[:, :], i32)
            lfidx_t = load("lfidx", (128, BS * 4), lfidx[:, :], i32)
            cidx_t = load("cidx", (128, BS), cidx[:, :], i32)
            sfidx_t = load("sfidx", (BS, 1), sfidx[:, :], i32)
            gct_t = load("gct", (BS, 1), gct[:, :])
            nout_t = load("nout", (BS, 1), nout[:, :])
            lfct_t = load("lfct", (O, BS), lfctT[:, :])
            fill_t = sb.tile([128, 8], f32, tag="fill")
            nc.vector.memset(fill_t[:], MASK_FILL)
            ninf_t = sb.tile([BS, O], f32, tag="ninf")
            nc.vector.memset(ninf_t[:], float("-inf"))

            def gather(table, idx_col, rows, width, tag, bufs):
                t = sb.tile([rows, width], f32, tag=tag, bufs=bufs)
                nc.gpsimd.indirect_dma_start(
                    out=t[:], out_offset=None, in_=table[:, :],
                    in_offset=bass.IndirectOffsetOnAxis(ap=idx_col, axis=0))
                return t

            # ---- gathers for encoder ----
            cen = gather(etab, sfidx_t[:, 0:1], BS, D, "cen", 1)
            cxs = [gather(etab, cidx_t[:, b:b + 1], 128, D, "cx", 3) for b in range(BS)]

            # ---- stage A: encoder ----
            cT = []
            for k in range(2):
                tps = ps.tile([128, BS], f32, tag="psml", bufs=4)
                nc.tensor.transpose(tps[:], cen[:, k * 128:(k + 1) * 128], id_t[0:BS, 0:BS])
                t = sb.tile([128, BS], f32, tag="cT", bufs=2)
                nc.vector.tensor_copy(t[:], tps[:])
                cT.append(t)
            cw_ps = ps.tile([BS, HID], f32, tag="psml", bufs=4)
            nc.tensor.matmul(cw_ps[:], lhsT=cT[0][:], rhs=fWt[0][:], start=True, stop=False)
            nc.tensor.matmul(cw_ps[:], lhsT=cT[1][:], rhs=fWt[1][:], start=False, stop=False)
            nc.tensor.matmul(cw_ps[:], lhsT=on_t[0:1, 0:BS], rhs=fb_t[:, :], start=False, stop=True)
            cwb = sb.tile([BS, HID], f32, tag="cwb")
            nc.vector.tensor_copy(cwb[:], cw_ps[:])
            cwrows = []
            for b in range(BS):
                r = sb.tile([1, HID], f32, tag="cwr", bufs=BS, name=f"cwr{b}")
                nc.sync.dma_start(r[:], cwb[b:b + 1, :])
                cwrows.append(r)

            plT_ps = [ps.tile([128, BS], f32, tag="plt", bufs=2, name=f"plTps{k}") for k in range(2)]
            for b in range(BS):
                cxT = []
                for k in range(2):
                    tps = ps.tile([128, 128], f32, tag="pbig", bufs=2)
                    nc.tensor.transpose(tps[:], cxs[b][:, k * 128:(k + 1) * 128], id_t[:, :])
                    t = sb.tile([128, 128], f32, tag="cxT", bufs=2)
                    nc.vector.tensor_copy(t[:], tps[:])
                    cxT.append(t)
                h_ps = ps.tile([128, HID], f32, tag="pbig", bufs=2)
                nc.tensor.matmul(h_ps[:], lhsT=cxT[0][:], rhs=fWt[2][:], start=True, stop=False)
                nc.tensor.matmul(h_ps[:], lhsT=cxT[1][:], rhs=fWt[3][:], start=False, stop=False)
                nc.tensor.matmul(h_ps[:], lhsT=on_t[0:1, :], rhs=cwrows[b][:], start=False, stop=True)
                hr = sb.tile([128, HID], f32, tag="hr", bufs=2)
                nc.scalar.activation(hr[:], h_ps[:], ACT.Relu)
                for k in range(2):
                    nc.tensor.matmul(plT_ps[k][:, b:b + 1],
                                     lhsT=hr[:, k * 128:(k + 1) * 128],
                                     rhs=on_t[:, 0:1], start=True, stop=True)
            plT = []
            for k in range(2):
                t = sb.tile([128, BS], f32, tag="plT", bufs=2)
                nc.scalar.mul(t[:], plT_ps[k][:], 1.0 / C)
                plT.append(t)
            uv_ps = ps.tile([BS, D + 1], f32, tag="psml", bufs=4)
            nc.tensor.matmul(uv_ps[:], lhsT=plT[0][:], rhs=uvt[0][:], start=True, stop=False)
            nc.tensor.matmul(uv_ps[:], lhsT=plT[1][:], rhs=uvt[1][:], start=False, stop=False)
            nc.tensor.matmul(uv_ps[:], lhsT=on_t[0:1, 0:BS], rhs=uvb_t[:, :], start=False, stop=True)

            psig = sb.tile([BS, 1], f32, tag="psig")
            nc.vector.tensor_copy(psig[:], uv_ps[:, D:D + 1])
            sqq = sb.tile([BS, 1], f32, tag="sqq")
            nc.scalar.activation(sqq[:], psig[:], ACT.Exp, scale=2.0)
            c1 = sb.tile([BS, 1], f32, tag="c1")
            nc.vector.tensor_scalar(c1[:], psig[:], -256.0, -128.0, op0=ALU.mult, op1=ALU.add)
            c2 = sb.tile([BS, 1], f32, tag="c2")
            nc.vector.tensor_scalar(c2[:], sqq[:], 128.0, None, op0=ALU.mult)
            bsrc = sb.tile([BS, D + 3], f32, tag="bsrc")
            nc.vector.tensor_copy(bsrc[:, 0:D], uv_ps[:, 0:D])
            nc.vector.tensor_copy(bsrc[:, D:D + 1], c1[:])
            nc.vector.tensor_copy(bsrc[:, D + 1:D + 2], c2[:])
            nc.vector.tensor_copy(bsrc[:, D + 2:D + 3], gct_t[:])

            nc.sync.dma_start(dbsrc[:, :], bsrc[:])
            score_cols = sb.tile([O, BS], f32, tag="scol")

            # ---- stages B & C per batch element ----
            SR = PR  # 264: gathered row stride inside the per-b big tiles
            nrm_all = sb.tile([O, BS], f32, tag="nrmall")
            nc.vector.tensor_scalar(nrm_all[:], lfct_t[:], 1.0, None, op0=ALU.max)
            rcn_all = sb.tile([O, BS], f32, tag="rcnall")
            nc.vector.reciprocal(rcn_all[:], nrm_all[:])
            qo_all = sb.tile([O, BS], f32, tag="qoall")
            sgn_all = sb.tile([O, BS], f32, tag="sgnall")

            def sview(ap, off, dims):
                return bass.AP(ap.tensor, ap.offset + off, [list(ap.ap[0])] + dims)

            for b in range(BS):
                bc_sb = sb.tile([128, D + 3], f32, tag="bcs", bufs=3)
                nc.sync.dma_start(
                    bc_sb[:], bass.AP(dbsrc[:, :].tensor, b * (D + 3), [[0, 128], [1, D + 3]]))
                gbig = sb.tile([128, 8 * SR], f32, tag="gb", bufs=3)
                for t in range(8):
                    nc.gpsimd.indirect_dma_start(
                        out=gbig[:, t * SR:(t + 1) * SR], out_offset=None, in_=ptab[:, :],
                        in_offset=bass.IndirectOffsetOnAxis(
                            ap=gidx_t[:, 8 * b + t:8 * b + t + 1], axis=0))
                lsgv = sview(gbig[:], D, [[SR, 8]])               # [128, 8] logsig view
                gmuv = sview(gbig[:], 0, [[SR, 8], [1, D]])       # [128, 8, 256]
                bcv = sview(bc_sb[:], 0, [[0, 8], [1, D]])        # bcast along tile dim
                dscb = sb.tile([128, 8, D], f32, tag="dscb", bufs=1)
                nc.vector.tensor_tensor(out=dscb[:], in0=gmuv, in1=bcv, op=ALU.subtract)
                quad = sb.tile([128, 8], f32, tag="quad", bufs=2)
                for t in range(8):
                    dsq = sb.tile([128, D], f32, tag="dsq", bufs=1)
                    nc.scalar.activation(dsq[:], dscb[:, t, :], ACT.Square,
                                         accum_out=quad[:, t:t + 1])
                dm1 = ps.tile([1, 1], f32, tag="plt", bufs=2, name=f"dm1_{b}")
                nc.tensor.matmul(dm1[:], lhsT=quad[:, 0:1], rhs=on_t[:, 0:1], start=True, stop=True)
                # kl on [128, 8]
                einv = sb.tile([128, 8], f32, tag="einv", bufs=2)
                nc.scalar.activation(einv[:], lsgv, ACT.Exp, scale=-2.0)
                t1 = sb.tile([128, 8], f32, tag="t1", bufs=2)
                nc.vector.tensor_scalar(t1[:], quad[:], 0.5, bc_sb[:, D + 1:D + 2],
                                        op0=ALU.mult, op1=ALU.add)
                t2 = sb.tile([128, 8], f32, tag="t2", bufs=2)
                nc.vector.tensor_mul(t2[:], t1[:], einv[:])
                t3 = sb.tile([128, 8], f32, tag="t3", bufs=2)
                nc.vector.tensor_scalar(t3[:], lsgv, 256.0, bc_sb[:, D:D + 1],
                                        op0=ALU.mult, op1=ALU.add)
                klb = sb.tile([128, 8], f32, tag="klb", bufs=2)
                nc.vector.tensor_add(klb[:], t2[:], t3[:])
                cmpm = sb.tile([128, 8], mybir.dt.uint32, tag="cmpm", bufs=2)
                nc.vector.tensor_tensor(
                    out=cmpm[:], in0=gio_t[:], in1=bc_sb[:, D + 2:D + 3].to_broadcast([128, 8]),
                    op=ALU.is_ge)
                nc.vector.copy_predicated(klb[:], cmpm[:], fill_t[:])
                # exact min over 1024
                mn1 = sb.tile([128, 1], f32, tag="mn1", bufs=2)
                nc.vector.tensor_reduce(mn1[:], klb[:], axis=mybir.AxisListType.X, op=ALU.min)
                mn_ps = ps.tile([1, 128], f32, tag="psml", bufs=4)
                nc.tensor.transpose(mn_ps[:], mn1[:], id_t[:, :])
                mns = sb.tile([1, 1], f32, tag="mns", bufs=2)
                nc.vector.tensor_reduce(mns[:], mn_ps[:], axis=mybir.AxisListType.X, op=ALU.min)
                # Match XLA/Eigen exp semantics: gradual subnormals, hard 0 at
                # x <= -97.2865 (measured cutoff, identical on cpu + neuron).
                # ACT's exp spline clamps before f32 underflow, so compute
                # exp(x/2)^2 and apply the cutoff mask explicitly.
                mrow = sb.tile([1, 2], f32, tag="mrow", bufs=2)
                nc.vector.tensor_scalar(mrow[:, 0:1], mns[:], 0.5, None, op0=ALU.mult)
                nc.vector.tensor_scalar(mrow[:, 1:2], mns[:], 1.0, 97.2865, op0=ALU.mult, op1=ALU.add)
                mnb_ps = ps.tile([128, 2], f32, tag="psml", bufs=4)
                nc.tensor.matmul(mnb_ps[:], lhsT=on_t[0:1, :], rhs=mrow[:], start=True, stop=True)
                mnb = sb.tile([128, 2], f32, tag="mnb", bufs=2)
                nc.vector.tensor_copy(mnb[:], mnb_ps[:])
                e_h = sb.tile([128, 8], f32, tag="eh", bufs=2)
                nc.scalar.activation(e_h[:], klb[:], ACT.Exp, scale=-0.5, bias=mnb[:, 0:1])
                e_sq = sb.tile([128, 8], f32, tag="esq", bufs=2)
                nc.vector.tensor_mul(e_sq[:], e_h[:], e_h[:])
                ecut = sb.tile([128, 8], f32, tag="ecut", bufs=2)
                nc.vector.tensor_scalar(ecut[:], klb[:], mnb[:, 1:2], None, op0=ALU.is_lt)
                e_b = sb.tile([128, 8], f32, tag="eb", bufs=2)
                nc.vector.tensor_mul(e_b[:], e_sq[:], ecut[:])
                # weighted posterior accumulate: [sum e*mu | sum e*ls | sum e]
                ws_ps = ps.tile([1, D + 2], f32, tag="psml", bufs=4)
                for t in range(8):
                    nc.tensor.matmul(ws_ps[:], lhsT=e_b[:, t:t + 1],
                                     rhs=gbig[:, t * SR:t * SR + D + 2],
                                     start=(t == 0), stop=(t == 7))
                dm2 = ps.tile([1, 1], f32, tag="plt", bufs=2, name=f"dm2_{b}")
                nc.tensor.matmul(dm2[:], lhsT=e_b[:, 0:1], rhs=on_t[:, 0:1], start=True, stop=True)
                sigx = sb.tile([128, 8], f32, tag="sigx", bufs=2)
                nc.scalar.activation(sigx[:], lsgv, ACT.Exp)
                scr2 = sb.tile([128, 8], f32, tag="scr2", bufs=2)
                pcol = sb.tile([128, 1], f32, tag="pcol", bufs=2)
                nc.vector.tensor_mul(scr2[:], e_b[:], sigx[:])
                nc.vector.tensor_reduce(pcol[:], scr2[:], axis=mybir.AxisListType.X, op=ALU.add)
                sg_ps = ps.tile([1, 1], f32, tag="psml", bufs=4)
                nc.tensor.matmul(sg_ps[:], lhsT=pcol[:], rhs=on_t[:, 0:1], start=True, stop=True)
                rc = sb.tile([1, 1], f32, tag="rc", bufs=2)
                nc.vector.reciprocal(rc[:], ws_ps[:, D + 1:D + 2])
                # posterior row: mu_post(256) | cc1 | cc2
                post = sb.tile([1, D + 2], f32, tag="post", bufs=2)
                nc.vector.tensor_scalar(post[:, 0:D], ws_ps[:, 0:D], rc[:], None, op0=ALU.mult)
                sgp = sb.tile([1, 1], f32, tag="sgp", bufs=2)
                nc.vector.tensor_scalar(sgp[:], sg_ps[:], rc[:], None, op0=ALU.mult)
                lnsg = sb.tile([1, 1], f32, tag="lnsg", bufs=2)
                nc.scalar.activation(lnsg[:], sgp[:], ACT.Ln)
                nc.vector.tensor_scalar(post[:, D:D + 1], lnsg[:], -256.0, -128.0,
                                        op0=ALU.mult, op1=ALU.add)
                sq2b = sb.tile([1, 1], f32, tag="sq2b", bufs=2)
                nc.vector.tensor_mul(sq2b[:], sgp[:], sgp[:])
                nc.vector.tensor_scalar(post[:, D + 1:D + 2], sq2b[:], 128.0, None, op0=ALU.mult)
                nc.sync.dma_start(ed_all[b, :, :], e_b[:])

                # ---- stage C (per-b part) ----
                nc.sync.dma_start(dpost[b:b + 1, :], post[:])
                bc2_sb = sb.tile([O, D + 2], f32, tag="bc2s", bufs=3)
                nc.sync.dma_start(
                    bc2_sb[:], bass.AP(dpost[:, :].tensor, b * (D + 2), [[0, O], [1, D + 2]]))
                lbig = sb.tile([128, 4 * SR], f32, tag="lb", bufs=2)
                for t in range(4):
                    nc.gpsimd.indirect_dma_start(
                        out=lbig[:, t * SR:(t + 1) * SR], out_offset=None, in_=ptab[:, :],
                        in_offset=bass.IndirectOffsetOnAxis(
                            ap=lfidx_t[:, 4 * b + t:4 * b + t + 1], axis=0))
                nc.scalar.activation(sview(lbig[:], D + 2, [[SR, 4]]),
                                     sview(lbig[:], D, [[SR, 4]]), ACT.Exp)
                lf_ps = ps.tile([O, D + 3], f32, tag="pbig", bufs=2)
                for t in range(4):
                    nc.tensor.matmul(lf_ps[:], lhsT=p4_t[t][:], rhs=lbig[:, t * SR:t * SR + D + 3],
                                     start=(t == 0), stop=(t == 3))
                d1 = sb.tile([O, D + 3], f32, tag="d1", bufs=2)
                nc.vector.tensor_scalar(d1[:], lf_ps[:], rcn_all[:, b:b + 1], None, op0=ALU.mult)
                dm3 = ps.tile([1, 1], f32, tag="plt", bufs=2, name=f"dm3_{b}")
                nc.tensor.matmul(dm3[:], lhsT=d1[:, 0:1], rhs=on_t[0:O, 0:1], start=True, stop=True)
                d2 = sb.tile([O, D], f32, tag="d2", bufs=2)
                nc.vector.tensor_sub(d2[:], d1[:, 0:D], bc2_sb[:, 0:D])
                d2sq = sb.tile([O, D], f32, tag="d2sq", bufs=2)
                nc.scalar.activation(d2sq[:], d2[:], ACT.Square, accum_out=qo_all[:, b:b + 1])
                nc.vector.tensor_copy(sgn_all[:, b:b + 1], d1[:, D + 2:D + 3])

            # ---- stage C batched tail ----
            cc1_all = sb.tile([O, BS], f32, tag="cc1all")
            nc.sync.dma_start(cc1_all[:], bass.AP(dpost[:, :].tensor, D, [[0, O], [D + 2, BS]]))
            cc2_all = sb.tile([O, BS], f32, tag="cc2all")
            nc.sync.dma_start(cc2_all[:], bass.AP(dpost[:, :].tensor, D + 1, [[0, O], [D + 2, BS]]))
            lns_a = sb.tile([O, BS], f32, tag="lnsa")
            nc.scalar.activation(lns_a[:], sgn_all[:], ACT.Ln)
            sq2_a = sb.tile([O, BS], f32, tag="sq2a")
            nc.vector.tensor_mul(sq2_a[:], sgn_all[:], sgn_all[:])
            isq_a = sb.tile([O, BS], f32, tag="isqa")
            nc.vector.reciprocal(isq_a[:], sq2_a[:])
            u1_a = sb.tile([O, BS], f32, tag="u1a")
            nc.vector.tensor_scalar(u1_a[:], qo_all[:], 0.5, None, op0=ALU.mult)
            nc.vector.tensor_add(u1_a[:], u1_a[:], cc2_all[:])
            u2_a = sb.tile([O, BS], f32, tag="u2a")
            nc.vector.tensor_mul(u2_a[:], u1_a[:], isq_a[:])
            u3_a = sb.tile([O, BS], f32, tag="u3a")
            nc.vector.tensor_scalar(u3_a[:], lns_a[:], 256.0, None, op0=ALU.mult)
            nc.vector.tensor_add(u3_a[:], u3_a[:], cc1_all[:])
            klo_a = sb.tile([O, BS], f32, tag="kloa")
            nc.vector.tensor_add(klo_a[:], u2_a[:], u3_a[:])
            nc.vector.tensor_scalar(score_cols[:], klo_a[:], -1.0, None, op0=ALU.mult)

            # ---- batched top-5 indices ----
            erow_all = sb.tile([BS, G], f32, tag="erowall")
            for b in range(BS):
                nc.sync.dma_start(erow_all[b:b + 1, :],
                                  ed_all[b, :, :].rearrange("p t -> t p"))
            t8 = sb.tile([BS, 8], f32, tag="t8")
            nc.vector.max(t8[:], erow_all[:])
            t8i = sb.tile([BS, 8], mybir.dt.uint32, tag="t8i")
            nc.vector.max_index(t8i[:], t8[:], erow_all[:])
            tw = sb.tile([BS, K_TOP], mybir.dt.int32, tag="tw")
            nc.vector.tensor_copy(tw[:], t8i[:, 0:K_TOP])
            nc.sync.dma_start(topw_out[:, :], tw[:])

            # ---- finalize score ----
            sc_ps = ps.tile([BS, O], f32, tag="psml", bufs=4)
            nc.tensor.transpose(sc_ps[:], score_cols[:], id_t[0:O, 0:O])
            scs = sb.tile([BS, O], f32, tag="scs")
            nc.vector.tensor_copy(scs[:], sc_ps[:])
            cmp2 = sb.tile([BS, O], mybir.dt.uint32, tag="cmp2")
            nc.vector.tensor_scalar(cmp2[:], io_t[:, 0:O], nout_t[:], None, op0=ALU.is_ge)
            nc.vector.copy_predicated(scs[:], cmp2[:], ninf_t[:])
            nc.sync.dma_start(score_out[:, :], scs[:])

    if split:
        split_multi_waits(nc, mybir)
    return nc


def host_prep(inputs):
    """Build per-core in_maps (layout/dtype transforms only)."""
    f = lambda x: np.ascontiguousarray(np.asarray(x), dtype=np.float32)
    i = lambda x: np.ascontiguousarray(np.asarray(x), dtype=np.int32)

    ptab = np.zeros((V, PR), np.float32)
    ptab[:, :D] = np.asarray(inputs["emb_mu"], np.float32)
    ptab[:, D] = np.asarray(inputs["emb_log_sigma"], np.float32)[:, 0]
    ptab[:, D + 1] = 1.0
    etab = f(inputs["enc_emb"])
    fW = f(inputs["f_W"])
    fb = f(inputs["f_b"]).reshape(1, HID)
    uvW = np.concatenate([f(inputs["u_W"]), f(inputs["v_W"])], axis=1)
    uvb = np.concatenate([f(inputs["u_b"]), f(inputs["v_b"])]).reshape(1, D + 1)
    ident = np.eye(128, dtype=np.float32)
    onesq = np.ones((128, 128), np.float32)
    pool4 = (np.arange(512)[:, None] // L == np.arange(O)[None, :]).astype(np.float32)
    iota_b = np.tile(np.arange(G, dtype=np.float32), (BS, 1))
    giota = (np.arange(8)[None, :] * 128 + np.arange(128)[:, None]).astype(np.float32)

    sf_ids = i(inputs["sf_ids"])
    context_ids = i(inputs["context_ids"])
    lf_ids = i(inputs["lf_ids"])
    global_ids = i(inputs["global_ids"])
    lf_token_ct = f(inputs["lf_token_ct"])
    gct_all = f(inputs["global_token_ct"]).reshape(B)
    nout_all = f(inputs["num_outputs"]).reshape(B)

    shared = dict(ptab=ptab, etab=etab, fW=fW, fb=fb, uvW=uvW, uvb=uvb,
                  ident=ident, onesq=onesq, pool4=pool4, iota_b=iota_b,
                  giota=giota)
    in_maps = []
    for c in range(NCORES):
        s = slice(c * BS, (c + 1) * BS)
        gi = global_ids[s].reshape(BS, 8, 128).transpose(2, 0, 1).reshape(128, BS * 8)
        li = lf_ids[s].reshape(BS, 512).reshape(BS, 4, 128).transpose(2, 0, 1).reshape(128, BS * 4)
        ci = context_ids[s].T
        in_maps.append(dict(
            shared,
            gidx=np.ascontiguousarray(gi), lfidx=np.ascontiguousarray(li),
            cidx=np.ascontiguousarray(ci),
            sfidx=np.ascontiguousarray(sf_ids[s].reshape(BS, 1)),
            gct=np.ascontiguousarray(gct_all[s].reshape(BS, 1)),
            nout=np.ascontiguousarray(nout_all[s].reshape(BS, 1)),
            lfctT=np.ascontiguousarray(lf_token_ct[s].T),
        ))
    return in_maps


def kernel(**inputs):
    from concourse.bass_utils import run_bass_kernel_spmd

    in_maps = host_prep(inputs)
    if _cache["nc"] is None:
        _cache["nc"] = build_nc()
    res = run_bass_kernel_spmd(_cache["nc"], in_maps, core_ids=list(range(NCORES)))
    _cache["last_res"] = res
    score = np.concatenate([r["score_out"] for r in res.results], axis=0)
    topw = np.concatenate([r["topw_out"] for r in res.results], axis=0)
    return score, np.asarray(inputs["target_lf_ids"]), topw


# revision 26
# speedup vs baseline: 1.2290x; 1.0388x over previous
"""Trainium2 Bass kernel for nn_AcronymExpander (topk_masking).

Data-parallel over batch: 8 NeuronCores x 8 batch elements each; embedding
tables replicated. All gathers via gpsimd indirect DMA; all direct DMA on
HWDGE (sync). Per-core pipeline:
  A) BSG encoder: ctx gather -> PE-transpose -> matmul -> relu -> mean pool
     -> (mu, log sigma) per batch element.
  B) KL attention over G=1024 global tokens: gather [mu|logsig|1] rows,
     quad via DVE sub + ACT square-accumulate, KL in [128,8] tiles,
     stable softmax via exact min (PE transpose trick), top-8 via DVE max8,
     attention-weighted posterior via PE matmul with an ones-column in the
     gathered rows providing the softmax denominator.
  C) Long-form scoring: gather LF rows, L-pooling via constant block-diagonal
     matmuls, KL against the posterior, positional masking to -inf.
"""
import sys
import types

import numpy as np

# ---- shim: antenv.axon_hooks is absent on this image; bass_utils imports it
# when tracing is requested (BASS_TRACE=1 or trace=True). Provide it so
# profiling works instead of crashing.
if "antenv.axon_hooks" not in sys.modules:
    _hook_mod = types.ModuleType("antenv.axon_hooks")
    _hook_state = {"h": None}
    _hook_mod.set_axon_ntff_profile_hook = lambda h: _hook_state.__setitem__("h", h)
    _hook_mod.get_axon_ntff_profile_hook = lambda: _hook_state["h"]
    sys.modules["antenv.axon_hooks"] = _hook_mod
    try:
        from trn_agent_boot.trn_boot import _ntff_profile_via_ctypes
        _hook_mod.set_axon_ntff_profile_hook(
            _ntff_profile_via_ctypes("/opt/axon/libaxon_pjrt.so"))
    except Exception:
        pass

B, C, O, L, G = 64, 128, 64, 8, 1024
V, D, HID = 50000, 256, 256
NCORES, BS = 8, 8           # batch shard per core
K_TOP = 5
MASK_FILL = 1e5
PR = 264                    # prior-table row: mu(256) | logsig | 1.0 | pad(6)

_cache = {"nc": None}


def split_multi_waits(nc, mybir):
    """This walrus rejects >1 sync wait per instruction: move extras to NoOps."""
    for f in nc.m.functions:
        for blk in f.blocks:
            out, changed = [], False
            for inst in blk.instructions:
                si = inst.sync_info
                if si is not None and si.on_wait is not None and len(si.on_wait) > 1:
                    waits = list(si.on_wait)
                    for j, w in enumerate(waits[:-1]):
                        n = mybir.InstNoOp(name=f"{inst.name}-w{j}", ins=[], outs=[])
                        n.engine = inst.engine
                        n.sync_info = mybir.SyncInfo(on_wait=[w], on_update=[])
                        out.append(n)
                    inst.sync_info = mybir.SyncInfo(
                        on_wait=[waits[-1]], on_update=list(si.on_update or []))
                    changed = True
                out.append(inst)
            if changed:
                blk.instructions = out


def build_nc(split=True):
    import concourse.bass as bass
    import concourse.mybir as mybir
    from concourse.tile import TileContext

    f32 = mybir.dt.float32
    i32 = mybir.dt.int32
    ALU = mybir.AluOpType
    ACT = mybir.ActivationFunctionType

    nc = bass.Bass(dynamic_dma_scratch_size=131072)
    P = lambda n, s, dt=f32: nc.declare_dram_parameter(n, list(s), dt, isOutput=False)
    ptab = P("ptab", (V, PR))
    etab = P("etab", (V, D))
    fW = P("fW", (2 * D, HID))
    fb = P("fb", (1, HID))
    uvW = P("uvW", (HID, D + 1))
    uvb = P("uvb", (1, D + 1))
    ident = P("ident", (128, 128))
    onesq = P("onesq", (128, 128))
    pool4 = P("pool4", (512, 64))
    iota_b = P("iota_b", (BS, G))
    giota = P("giota", (128, 8))
    gidx = P("gidx", (128, BS * 8), i32)
    lfidx = P("lfidx", (128, BS * 4), i32)
    cidx = P("cidx", (128, BS), i32)
    sfidx = P("sfidx", (BS, 1), i32)
    gct = P("gct", (BS, 1))
    nout = P("nout", (BS, 1))
    lfctT = P("lfctT", (O, BS))
    score_out = nc.declare_dram_parameter("score_out", [BS, O], f32, isOutput=True)
    topw_out = nc.declare_dram_parameter("topw_out", [BS, K_TOP], mybir.dt.int32, isOutput=True)
    ed_all = nc.dram_tensor("ed_all", [BS, 128, 8], f32)
    dbsrc = nc.dram_tensor("dbsrc", [BS, D + 3], f32)
    dpost = nc.dram_tensor("dpost", [BS, D + 2], f32)

    with TileContext(nc) as tc:
        with (
            tc.tile_pool(name="sb", bufs=1) as sb,
            tc.tile_pool(name="ps", space="PSUM", bufs=1) as ps,
        ):
            # ---- persistent constants / weights ----
            def load(name, shape, src, dt=f32):
                t = sb.tile(list(shape), dt, tag=name)
                nc.sync.dma_start(t[:], src)
                return t

            fWt = [load(f"fw{k}", (128, HID), fW[k * 128:(k + 1) * 128, :]) for k in range(4)]
            uvt = [load(f"uv{k}", (128, D + 1), uvW[k * 128:(k + 1) * 128, :]) for k in range(2)]
            fb_t = load("fb", (1, HID), fb[:, :])
            uvb_t = load("uvb", (1, D + 1), uvb[:, :])
            id_t = load("id", (128, 128), ident[:, :])
            on_t = load("on", (128, 128), onesq[:, :])
            p4_t = [load(f"p4{t}", (128, 64), pool4[t * 128:(t + 1) * 128, :]) for t in range(4)]
            io_t = load("io", (BS, G), iota_b[:, :])
            gio_t = load("gio", (128, 8), giota[:, :])
            gidx_t = load("gidx", (128, BS * 8), gidx[:, :], i32)
            lfidx_t = load("lfidx", (128, BS * 4), lfidx[:, :], i32)
            cidx_t = load("cidx", (128, BS), cidx[:, :], i32)
            sfidx_t = load("sfidx", (BS, 1), sfidx[:, :], i32)
            gct_t = load("gct", (BS, 1), gct[:, :])
            nout_t = load("nout", (BS, 1), nout[:, :])
            lfct_t = load("lfct", (O, BS), lfctT[:, :])
            fill_t = sb.tile([128, 8], f32, tag="fill")
            nc.vector.memset(fill_t[:], MASK_FILL)
            ninf_t = sb.tile([BS, O], f32, tag="ninf")
            nc.vector.memset(ninf_t[:], float("-inf"))

            def gather(table, idx_col, rows, width, tag, bufs):
                t = sb.tile([rows, width], f32, tag=tag, bufs=bufs)
                nc.gpsimd.indirect_dma_start(
                    out=t[:], out_offset=None, in_=table[:, :],
                    in_offset=bass.IndirectOffsetOnAxis(ap=idx_col, axis=0))
                return t

            # ---- gathers for encoder ----
            cen = gather(etab, sfidx_t[:, 0:1], BS, D, "cen", 1)
            cxs = [gather(etab, cidx_t[:, b:b + 1], 128, D, "cx", 3) for b in range(BS)]

            # ---- stage A: encoder ----
            cT = []
            for k in range(2):
                tps = ps.tile([128, BS], f32, tag="psml", bufs=4)
                nc.tensor.transpose(tps[:], cen[:, k * 128:(k + 1) * 128], id_t[0:BS, 0:BS])
                t = sb.tile([128, BS], f32, tag="cT", bufs=2)
                nc.vector.tensor_copy(t[:], tps[:])
                cT.append(t)
            cw_ps = ps.tile([BS, HID], f32, tag="psml", bufs=4)
            nc.tensor.matmul(cw_ps[:], lhsT=cT[0][:], rhs=fWt[0][:], start=True, stop=False)
            nc.tensor.matmul(cw_ps[:], lhsT=cT[1][:], rhs=fWt[1][:], start=False, stop=False)
            nc.tensor.matmul(cw_ps[:], lhsT=on_t[0:1, 0:BS], rhs=fb_t[:, :], start=False, stop=True)
            cwb = sb.tile([BS, HID], f32, tag="cwb")
            nc.vector.tensor_copy(cwb[:], cw_ps[:])
            cwrows = []
            for b in range(BS):
                r = sb.tile([1, HID], f32, tag="cwr", bufs=BS, name=f"cwr{b}")
                nc.sync.dma_start(r[:], cwb[b:b + 1, :])
                cwrows.append(r)

            plT_ps = [ps.tile([128, BS], f32, tag="plt", bufs=2, name=f"plTps{k}") for k in range(2)]
            for b in range(BS):
                cxT = []
                for k in range(2):
                    tps = ps.tile([128, 128], f32, tag="pbig", bufs=2)
                    nc.tensor.transpose(tps[:], cxs[b][:, k * 128:(k + 1) * 128], id_t[:, :])
                    t = sb.tile([128, 128], f32, tag="cxT", bufs=2)
                    nc.vector.tensor_copy(t[:], tps[:])
                    cxT.append(t)
                h_ps = ps.tile([128, HID], f32, tag="pbig", bufs=2)
                nc.tensor.matmul(h_ps[:], lhsT=cxT[0][:], rhs=fWt[2][:], start=True, stop=False)
                nc.tensor.matmul(h_ps[:], lhsT=cxT[1][:], rhs=fWt[3][:], start=False, stop=False)
                nc.tensor.matmul(h_ps[:], lhsT=on_t[0:1, :], rhs=cwrows[b][:], start=False, stop=True)
                hr = sb.tile([128, HID], f32, tag="hr", bufs=2)
                nc.scalar.activation(hr[:], h_ps[:], ACT.Relu)
                for k in range(2):
                    nc.tensor.matmul(plT_ps[k][:, b:b + 1],
                                     lhsT=hr[:, k * 128:(k + 1) * 128],
                                     rhs=on_t[:, 0:1], start=True, stop=True)
            plT = []
            for k in range(2):
                t = sb.tile([128, BS], f32, tag="plT", bufs=2)
                nc.scalar.mul(t[:], plT_ps[k][:], 1.0 / C)
                plT.append(t)
            uv_ps = ps.tile([BS, D + 1], f32, tag="psml", bufs=4)
            nc.tensor.matmul(uv_ps[:], lhsT=plT[0][:], rhs=uvt[0][:], start=True, stop=False)
            nc.tensor.matmul(uv_ps[:], lhsT=plT[1][:], rhs=uvt[1][:], start=False, stop=False)
            nc.tensor.matmul(uv_ps[:], lhsT=on_t[0:1, 0:BS], rhs=uvb_t[:, :], start=False, stop=True)

            psig = sb.tile([BS, 1], f32, tag="psig")
            nc.vector.tensor_copy(psig[:], uv_ps[:, D:D + 1])
            sqq = sb.tile([BS, 1], f32, tag="sqq")
            nc.scalar.activation(sqq[:], psig[:], ACT.Exp, scale=2.0)
            c1 = sb.tile([BS, 1], f32, tag="c1")
            nc.vector.tensor_scalar(c1[:], psig[:], -256.0, -128.0, op0=ALU.mult, op1=ALU.add)
            c2 = sb.tile([BS, 1], f32, tag="c2")
            nc.vector.tensor_scalar(c2[:], sqq[:], 128.0, None, op0=ALU.mult)
            bsrc = sb.tile([BS, D + 3], f32, tag="bsrc")
            nc.vector.tensor_copy(bsrc[:, 0:D], uv_ps[:, 0:D])
            nc.vector.tensor_copy(bsrc[:, D:D + 1], c1[:])
            nc.vector.tensor_copy(bsrc[:, D + 1:D + 2], c2[:])
            nc.vector.tensor_copy(bsrc[:, D + 2:D + 3], gct_t[:])

            nc.sync.dma_start(dbsrc[:, :], bsrc[:])
            score_cols = sb.tile([O, BS], f32, tag="scol")

            # ---- stages B & C per batch element ----
            SR = PR  # 264: gathered row stride inside the per-b big tiles
            nrm_all = sb.tile([O, BS], f32, tag="nrmall")
            nc.vector.tensor_scalar(nrm_all[:], lfct_t[:], 1.0, None, op0=ALU.max)
            rcn_all = sb.tile([O, BS], f32, tag="rcnall")
            nc.vector.reciprocal(rcn_all[:], nrm_all[:])
            qo_all = sb.tile([O, BS], f32, tag="qoall")
            sgn_all = sb.tile([O, BS], f32, tag="sgnall")

            def sview(ap, off, dims):
                return bass.AP(ap.tensor, ap.offset + off, [list(ap.ap[0])] + dims)

            for b in range(BS):
                bc_sb = sb.tile([128, D + 3], f32, tag="bcs", bufs=3)
                nc.sync.dma_start(
                    bc_sb[:], bass.AP(dbsrc[:, :].tensor, b * (D + 3), [[0, 128], [1, D + 3]]))
                gbig = sb.tile([128, 8 * SR], f32, tag="gb", bufs=3)
                for t in range(8):
                    nc.gpsimd.indirect_dma_start(
                        out=gbig[:, t * SR:(t + 1) * SR], out_offset=None, in_=ptab[:, :],
                        in_offset=bass.IndirectOffsetOnAxis(
                            ap=gidx_t[:, 8 * b + t:8 * b + t + 1], axis=0))
                lsgv = sview(gbig[:], D, [[SR, 8]])               # [128, 8] logsig view
                gmuv = sview(gbig[:], 0, [[SR, 8], [1, D]])       # [128, 8, 256]
                bcv = sview(bc_sb[:], 0, [[0, 8], [1, D]])        # bcast along tile dim
                quad = sb.tile([128, 8], f32, tag="quad", bufs=2)
                for h in range(2):
                    gmuh = sview(gbig[:], 4 * h * SR, [[SR, 4], [1, D]])
                    bch = sview(bc_sb[:], 0, [[0, 4], [1, D]])
                    dscb = sb.tile([128, 4, D], f32, tag="dscb", bufs=2, name=f"dscb{h}")
                    nc.vector.tensor_tensor(out=dscb[:], in0=gmuh, in1=bch, op=ALU.subtract)
                    for t in range(4):
                        dsq = sb.tile([128, D], f32, tag="dsq", bufs=1)
                        nc.scalar.activation(dsq[:], dscb[:, t, :], ACT.Square,
                                             accum_out=quad[:, 4 * h + t:4 * h + t + 1])
                # kl on [128, 8]
                einv = sb.tile([128, 8], f32, tag="einv", bufs=2)
                nc.scalar.activation(einv[:], lsgv, ACT.Exp, scale=-2.0)
                t1 = sb.tile([128, 8], f32, tag="t1", bufs=2)
                nc.vector.tensor_scalar(t1[:], quad[:], 0.5, bc_sb[:, D + 1:D + 2],
                                        op0=ALU.mult, op1=ALU.add)
                t2 = sb.tile([128, 8], f32, tag="t2", bufs=2)
                nc.vector.tensor_mul(t2[:], t1[:], einv[:])
                t3 = sb.tile([128, 8], f32, tag="t3", bufs=2)
                nc.vector.tensor_scalar(t3[:], lsgv, 256.0, bc_sb[:, D:D + 1],
                                        op0=ALU.mult, op1=ALU.add)
                klb = sb.tile([128, 8], f32, tag="klb", bufs=2)
                nc.vector.tensor_add(klb[:], t2[:], t3[:])
                cmpm = sb.tile([128, 8], mybir.dt.uint32, tag="cmpm", bufs=2)
                nc.vector.tensor_tensor(
                    out=cmpm[:], in0=gio_t[:], in1=bc_sb[:, D + 2:D + 3].to_broadcast([128, 8]),
                    op=ALU.is_ge)
                nc.vector.copy_predicated(klb[:], cmpm[:], fill_t[:])
                # exact min over 1024
                mn1 = sb.tile([128, 1], f32, tag="mn1", bufs=2)
                nc.vector.tensor_reduce(mn1[:], klb[:], axis=mybir.AxisListType.X, op=ALU.min)
                mn_ps = ps.tile([1, 128], f32, tag="psml", bufs=4)
                nc.tensor.transpose(mn_ps[:], mn1[:], id_t[:, :])
                mns = sb.tile([1, 1], f32, tag="mns", bufs=2)
                nc.vector.tensor_reduce(mns[:], mn_ps[:], axis=mybir.AxisListType.X, op=ALU.min)
                # Match XLA/Eigen exp semantics: gradual subnormals, hard 0 at
                # x <= -97.2865 (measured cutoff, identical on cpu + neuron).
                # ACT's exp spline clamps before f32 underflow, so compute
                # exp(x/2)^2 and apply the cutoff mask explicitly.
                mrow = sb.tile([1, 2], f32, tag="mrow", bufs=2)
                nc.vector.tensor_scalar(mrow[:, 0:1], mns[:], 0.5, None, op0=ALU.mult)
                nc.vector.tensor_scalar(mrow[:, 1:2], mns[:], 1.0, 97.2865, op0=ALU.mult, op1=ALU.add)
                mnb_ps = ps.tile([128, 2], f32, tag="psml", bufs=4)
                nc.tensor.matmul(mnb_ps[:], lhsT=on_t[0:1, :], rhs=mrow[:], start=True, stop=True)
                mnb = sb.tile([128, 2], f32, tag="mnb", bufs=2)
                nc.vector.tensor_copy(mnb[:], mnb_ps[:])
                e_h = sb.tile([128, 8], f32, tag="eh", bufs=2)
                nc.scalar.activation(e_h[:], klb[:], ACT.Exp, scale=-0.5, bias=mnb[:, 0:1])
                e_sq = sb.tile([128, 8], f32, tag="esq", bufs=2)
                nc.vector.tensor_mul(e_sq[:], e_h[:], e_h[:])
                ecut = sb.tile([128, 8], f32, tag="ecut", bufs=2)
                nc.vector.tensor_scalar(ecut[:], klb[:], mnb[:, 1:2], None, op0=ALU.is_lt)
                e_b = sb.tile([128, 8], f32, tag="eb", bufs=2)
                nc.vector.tensor_mul(e_b[:], e_sq[:], ecut[:])
                # weighted posterior accumulate: [sum e*mu | sum e*ls | sum e]
                ws_ps = ps.tile([1, D + 2], f32, tag="psml", bufs=4)
                for t in range(8):
                    nc.tensor.matmul(ws_ps[:], lhsT=e_b[:, t:t + 1],
                                     rhs=gbig[:, t * SR:t * SR + D + 2],
                                     start=(t == 0), stop=(t == 7))
                sigx = sb.tile([128, 8], f32, tag="sigx", bufs=2)
                nc.scalar.activation(sigx[:], lsgv, ACT.Exp)
                scr2 = sb.tile([128, 8], f32, tag="scr2", bufs=2)
                pcol = sb.tile([128, 1], f32, tag="pcol", bufs=2)
                nc.vector.tensor_mul(scr2[:], e_b[:], sigx[:])
                nc.vector.tensor_reduce(pcol[:], scr2[:], axis=mybir.AxisListType.X, op=ALU.add)
                sg_ps = ps.tile([1, 1], f32, tag="psml", bufs=4)
                nc.tensor.matmul(sg_ps[:], lhsT=pcol[:], rhs=on_t[:, 0:1], start=True, stop=True)
                rc = sb.tile([1, 1], f32, tag="rc", bufs=2)
                nc.vector.reciprocal(rc[:], ws_ps[:, D + 1:D + 2])
                # posterior row: mu_post(256) | cc1 | cc2
                post = sb.tile([1, D + 2], f32, tag="post", bufs=2)
                nc.vector.tensor_scalar(post[:, 0:D], ws_ps[:, 0:D], rc[:], None, op0=ALU.mult)
                sgp = sb.tile([1, 1], f32, tag="sgp", bufs=2)
                nc.vector.tensor_scalar(sgp[:], sg_ps[:], rc[:], None, op0=ALU.mult)
                lnsg = sb.tile([1, 1], f32, tag="lnsg", bufs=2)
                nc.scalar.activation(lnsg[:], sgp[:], ACT.Ln)
                nc.vector.tensor_scalar(post[:, D:D + 1], lnsg[:], -256.0, -128.0,
                                        op0=ALU.mult, op1=ALU.add)
                sq2b = sb.tile([1, 1], f32, tag="sq2b", bufs=2)
                nc.vector.tensor_mul(sq2b[:], sgp[:], sgp[:])
                nc.vector.tensor_scalar(post[:, D + 1:D + 2], sq2b[:], 128.0, None, op0=ALU.mult)
                nc.sync.dma_start(ed_all[b, :, :], e_b[:])

                # ---- stage C (per-b part) ----
                nc.sync.dma_start(dpost[b:b + 1, :], post[:])
                bc2_sb = sb.tile([O, D + 2], f32, tag="bc2s", bufs=3)
                nc.sync.dma_start(
                    bc2_sb[:], bass.AP(dpost[:, :].tensor, b * (D + 2), [[0, O], [1, D + 2]]))
                lbig = sb.tile([128, 4 * SR], f32, tag="lb", bufs=2)
                for t in range(4):
                    nc.gpsimd.indirect_dma_start(
                        out=lbig[:, t * SR:(t + 1) * SR], out_offset=None, in_=ptab[:, :],
                        in_offset=bass.IndirectOffsetOnAxis(
                            ap=lfidx_t[:, 4 * b + t:4 * b + t + 1], axis=0))
                nc.scalar.activation(sview(lbig[:], D + 2, [[SR, 4]]),
                                     sview(lbig[:], D, [[SR, 4]]), ACT.Exp)
                lf_ps = ps.tile([O, D + 3], f32, tag="pbig", bufs=2)
                for t in range(4):
                    nc.tensor.matmul(lf_ps[:], lhsT=p4_t[t][:], rhs=lbig[:, t * SR:t * SR + D + 3],
                                     start=(t == 0), stop=(t == 3))
                d1 = sb.tile([O, D + 3], f32, tag="d1", bufs=2)
                nc.vector.tensor_scalar(d1[:], lf_ps[:], rcn_all[:, b:b + 1], None, op0=ALU.mult)
                d2 = sb.tile([O, D], f32, tag="d2", bufs=2)
                nc.vector.tensor_sub(d2[:], d1[:, 0:D], bc2_sb[:, 0:D])
                d2sq = sb.tile([O, D], f32, tag="d2sq", bufs=2)
                nc.scalar.activation(d2sq[:], d2[:], ACT.Square, accum_out=qo_all[:, b:b + 1])
                nc.vector.tensor_copy(sgn_all[:, b:b + 1], d1[:, D + 2:D + 3])

            # ---- stage C batched tail ----
            cc1_all = sb.tile([O, BS], f32, tag="cc1all")
            nc.sync.dma_start(cc1_all[:], bass.AP(dpost[:, :].tensor, D, [[0, O], [D + 2, BS]]))
            cc2_all = sb.tile([O, BS], f32, tag="cc2all")
            nc.sync.dma_start(cc2_all[:], bass.AP(dpost[:, :].tensor, D + 1, [[0, O], [D + 2, BS]]))
            lns_a = sb.tile([O, BS], f32, tag="lnsa")
            nc.scalar.activation(lns_a[:], sgn_all[:], ACT.Ln)
            sq2_a = sb.tile([O, BS], f32, tag="sq2a")
            nc.vector.tensor_mul(sq2_a[:], sgn_all[:], sgn_all[:])
            isq_a = sb.tile([O, BS], f32, tag="isqa")
            nc.vector.reciprocal(isq_a[:], sq2_a[:])
            u1_a = sb.tile([O, BS], f32, tag="u1a")
            nc.vector.tensor_scalar(u1_a[:], qo_all[:], 0.5, None, op0=ALU.mult)
            nc.vector.tensor_add(u1_a[:], u1_a[:], cc2_all[:])
            u2_a = sb.tile([O, BS], f32, tag="u2a")
            nc.vector.tensor_mul(u2_a[:], u1_a[:], isq_a[:])
            u3_a = sb.tile([O, BS], f32, tag="u3a")
            nc.vector.tensor_scalar(u3_a[:], lns_a[:], 256.0, None, op0=ALU.mult)
            nc.vector.tensor_add(u3_a[:], u3_a[:], cc1_all[:])
            klo_a = sb.tile([O, BS], f32, tag="kloa")
            nc.vector.tensor_add(klo_a[:], u2_a[:], u3_a[:])
            nc.vector.tensor_scalar(score_cols[:], klo_a[:], -1.0, None, op0=ALU.mult)

            # ---- batched top-5 indices ----
            erow_all = sb.tile([BS, G], f32, tag="erowall")
            for b in range(BS):
                nc.sync.dma_start(erow_all[b:b + 1, :],
                                  ed_all[b, :, :].rearrange("p t -> t p"))
            t8 = sb.tile([BS, 8], f32, tag="t8")
            nc.vector.max(t8[:], erow_all[:])
            t8i = sb.tile([BS, 8], mybir.dt.uint32, tag="t8i")
            nc.vector.max_index(t8i[:], t8[:], erow_all[:])
            tw = sb.tile([BS, K_TOP], mybir.dt.int32, tag="tw")
            nc.vector.tensor_copy(tw[:], t8i[:, 0:K_TOP])
            nc.sync.dma_start(topw_out[:, :], tw[:])

            # ---- finalize score ----
            sc_ps = ps.tile([BS, O], f32, tag="psml", bufs=4)
            nc.tensor.transpose(sc_ps[:], score_cols[:], id_t[0:O, 0:O])
            scs = sb.tile([BS, O], f32, tag="scs")
            nc.vector.tensor_copy(scs[:], sc_ps[:])
            cmp2 = sb.tile([BS, O], mybir.dt.uint32, tag="cmp2")
            nc.vector.tensor_scalar(cmp2[:], io_t[:, 0:O], nout_t[:], None, op0=ALU.is_ge)
            nc.vector.copy_predicated(scs[:], cmp2[:], ninf_t[:])
            nc.sync.dma_start(score_out[:, :], scs[:])

    if split:
        split_multi_waits(nc, mybir)
    return nc


def host_prep(inputs):
    """Build per-core in_maps (layout/dtype transforms only)."""
    f = lambda x: np.ascontiguousarray(np.asarray(x), dtype=np.float32)
    i = lambda x: np.ascontiguousarray(np.asarray(x), dtype=np.int32)

    ptab = np.zeros((V, PR), np.float32)
    ptab[:, :D] = np.asarray(inputs["emb_mu"], np.float32)
    ptab[:, D] = np.asarray(inputs["emb_log_sigma"], np.float32)[:, 0]
    ptab[:, D + 1] = 1.0
    etab = f(inputs["enc_emb"])
    fW = f(inputs["f_W"])
    fb = f(inputs["f_b"]).reshape(1, HID)
    uvW = np.concatenate([f(inputs["u_W"]), f(inputs["v_W"])], axis=1)
    uvb = np.concatenate([f(inputs["u_b"]), f(inputs["v_b"])]).reshape(1, D + 1)
    ident = np.eye(128, dtype=np.float32)
    onesq = np.ones((128, 128), np.float32)
    pool4 = (np.arange(512)[:, None] // L == np.arange(O)[None, :]).astype(np.float32)
    iota_b = np.tile(np.arange(G, dtype=np.float32), (BS, 1))
    giota = (np.arange(8)[None, :] * 128 + np.arange(128)[:, None]).astype(np.float32)

    sf_ids = i(inputs["sf_ids"])
    context_ids = i(inputs["context_ids"])
    lf_ids = i(inputs["lf_ids"])
    global_ids = i(inputs["global_ids"])
    lf_token_ct = f(inputs["lf_token_ct"])
    gct_all = f(inputs["global_token_ct"]).reshape(B)
    nout_all = f(inputs["num_outputs"]).reshape(B)

    shared = dict(ptab=ptab, etab=etab, fW=fW, fb=fb, uvW=uvW, uvb=uvb,
                  ident=ident, onesq=onesq, pool4=pool4, iota_b=iota_b,
                  giota=giota)
    in_maps = []
    for c in range(NCORES):
        s = slice(c * BS, (c + 1) * BS)
        gi = global_ids[s].reshape(BS, 8, 128).transpose(2, 0, 1).reshape(128, BS * 8)
        li = lf_ids[s].reshape(BS, 512).reshape(BS, 4, 128).transpose(2, 0, 1).reshape(128, BS * 4)
        ci = context_ids[s].T
        in_maps.append(dict(
            shared,
            gidx=np.ascontiguousarray(gi), lfidx=np.ascontiguousarray(li),
            cidx=np.ascontiguousarray(ci),
            sfidx=np.ascontiguousarray(sf_ids[s].reshape(BS, 1)),
            gct=np.ascontiguousarray(gct_all[s].reshape(BS, 1)),
            nout=np.ascontiguousarray(nout_all[s].reshape(BS, 1)),
            lfctT=np.ascontiguousarray(lf_token_ct[s].T),
        ))
    return in_maps


def kernel(**inputs):
    from concourse.bass_utils import run_bass_kernel_spmd

    in_maps = host_prep(inputs)
    if _cache["nc"] is None:
        _cache["nc"] = build_nc()
    res = run_bass_kernel_spmd(_cache["nc"], in_maps, core_ids=list(range(NCORES)))
    _cache["last_res"] = res
    score = np.concatenate([r["score_out"] for r in res.results], axis=0)
    topw = np.concatenate([r["topw_out"] for r in res.results], axis=0)
    return score, np.asarray(inputs["target_lf_ids"]), topw


# revision 27
# speedup vs baseline: 1.2545x; 1.0207x over previous
"""Trainium2 Bass kernel for nn_AcronymExpander (topk_masking).

Data-parallel over batch: 8 NeuronCores x 8 batch elements each; embedding
tables replicated. All gathers via gpsimd indirect DMA; all direct DMA on
HWDGE (sync). Per-core pipeline:
  A) BSG encoder: ctx gather -> PE-transpose -> matmul -> relu -> mean pool
     -> (mu, log sigma) per batch element.
  B) KL attention over G=1024 global tokens: gather [mu|logsig|1] rows,
     quad via DVE sub + ACT square-accumulate, KL in [128,8] tiles,
     stable softmax via exact min (PE transpose trick), top-8 via DVE max8,
     attention-weighted posterior via PE matmul with an ones-column in the
     gathered rows providing the softmax denominator.
  C) Long-form scoring: gather LF rows, L-pooling via constant block-diagonal
     matmuls, KL against the posterior, positional masking to -inf.
"""
import sys
import types

import numpy as np

# ---- shim: antenv.axon_hooks is absent on this image; bass_utils imports it
# when tracing is requested (BASS_TRACE=1 or trace=True). Provide it so
# profiling works instead of crashing.
if "antenv.axon_hooks" not in sys.modules:
    _hook_mod = types.ModuleType("antenv.axon_hooks")
    _hook_state = {"h": None}
    _hook_mod.set_axon_ntff_profile_hook = lambda h: _hook_state.__setitem__("h", h)
    _hook_mod.get_axon_ntff_profile_hook = lambda: _hook_state["h"]
    sys.modules["antenv.axon_hooks"] = _hook_mod
    try:
        from trn_agent_boot.trn_boot import _ntff_profile_via_ctypes
        _hook_mod.set_axon_ntff_profile_hook(
            _ntff_profile_via_ctypes("/opt/axon/libaxon_pjrt.so"))
    except Exception:
        pass

B, C, O, L, G = 64, 128, 64, 8, 1024
V, D, HID = 50000, 256, 256
NCORES, BS = 8, 8           # batch shard per core
K_TOP = 5
MASK_FILL = 1e5
PR = 264                    # prior-table row: mu(256) | logsig | 1.0 | pad(6)

_cache = {"nc": None}


def split_multi_waits(nc, mybir):
    """This walrus rejects >1 sync wait per instruction: move extras to NoOps."""
    for f in nc.m.functions:
        for blk in f.blocks:
            out, changed = [], False
            for inst in blk.instructions:
                si = inst.sync_info
                if si is not None and si.on_wait is not None and len(si.on_wait) > 1:
                    waits = list(si.on_wait)
                    for j, w in enumerate(waits[:-1]):
                        n = mybir.InstNoOp(name=f"{inst.name}-w{j}", ins=[], outs=[])
                        n.engine = inst.engine
                        n.sync_info = mybir.SyncInfo(on_wait=[w], on_update=[])
                        out.append(n)
                    inst.sync_info = mybir.SyncInfo(
                        on_wait=[waits[-1]], on_update=list(si.on_update or []))
                    changed = True
                out.append(inst)
            if changed:
                blk.instructions = out


def build_nc(split=True):
    import concourse.bass as bass
    import concourse.mybir as mybir
    from concourse.tile import TileContext

    f32 = mybir.dt.float32
    i32 = mybir.dt.int32
    ALU = mybir.AluOpType
    ACT = mybir.ActivationFunctionType

    nc = bass.Bass(dynamic_dma_scratch_size=98304)
    P = lambda n, s, dt=f32: nc.declare_dram_parameter(n, list(s), dt, isOutput=False)
    ptab = P("ptab", (V, PR))
    etab = P("etab", (V, D))
    fW = P("fW", (2 * D, HID))
    fb = P("fb", (1, HID))
    uvW = P("uvW", (HID, D + 1))
    uvb = P("uvb", (1, D + 1))
    ident = P("ident", (128, 128))
    onesq = P("onesq", (128, 128))
    pool4 = P("pool4", (512, 64))
    iota_b = P("iota_b", (BS, G))
    giota = P("giota", (128, 8))
    gidx = P("gidx", (128, BS * 8), i32)
    lfidx = P("lfidx", (128, BS * 4), i32)
    cidx = P("cidx", (128, BS), i32)
    sfidx = P("sfidx", (BS, 1), i32)
    gct = P("gct", (BS, 1))
    nout = P("nout", (BS, 1))
    lfctT = P("lfctT", (O, BS))
    score_out = nc.declare_dram_parameter("score_out", [BS, O], f32, isOutput=True)
    topw_out = nc.declare_dram_parameter("topw_out", [BS, K_TOP], mybir.dt.int32, isOutput=True)
    ed_all = nc.dram_tensor("ed_all", [BS, 128, 8], f32)
    dbsrc = nc.dram_tensor("dbsrc", [BS, D + 3], f32)
    dpost = nc.dram_tensor("dpost", [BS, D + 2], f32)

    with TileContext(nc) as tc:
        with (
            tc.tile_pool(name="sb", bufs=1) as sb,
            tc.tile_pool(name="ps", space="PSUM", bufs=1) as ps,
        ):
            # ---- persistent constants / weights ----
            def load(name, shape, src, dt=f32):
                t = sb.tile(list(shape), dt, tag=name)
                nc.sync.dma_start(t[:], src)
                return t

            fWt = [load(f"fw{k}", (128, HID), fW[k * 128:(k + 1) * 128, :]) for k in range(4)]
            uvt = [load(f"uv{k}", (128, D + 1), uvW[k * 128:(k + 1) * 128, :]) for k in range(2)]
            fb_t = load("fb", (1, HID), fb[:, :])
            uvb_t = load("uvb", (1, D + 1), uvb[:, :])
            id_t = load("id", (128, 128), ident[:, :])
            on_t = load("on", (128, 128), onesq[:, :])
            p4_t = [load(f"p4{t}", (128, 64), pool4[t * 128:(t + 1) * 128, :]) for t in range(4)]
            io_t = load("io", (BS, G), iota_b[:, :])
            gio_t = load("gio", (128, 8), giota[:, :])
            gidx_t = load("gidx", (128, BS * 8), gidx[:, :], i32)
            lfidx_t = load("lfidx", (128, BS * 4), lfidx[:, :], i32)
            cidx_t = load("cidx", (128, BS), cidx[:, :], i32)
            sfidx_t = load("sfidx", (BS, 1), sfidx[:, :], i32)
            gct_t = load("gct", (BS, 1), gct[:, :])
            nout_t = load("nout", (BS, 1), nout[:, :])
            lfct_t = load("lfct", (O, BS), lfctT[:, :])
            fill_t = sb.tile([128, 8], f32, tag="fill")
            nc.vector.memset(fill_t[:], MASK_FILL)
            ninf_t = sb.tile([BS, O], f32, tag="ninf")
            nc.vector.memset(ninf_t[:], float("-inf"))

            def gather(table, idx_col, rows, width, tag, bufs):
                t = sb.tile([rows, width], f32, tag=tag, bufs=bufs)
                nc.gpsimd.indirect_dma_start(
                    out=t[:], out_offset=None, in_=table[:, :],
                    in_offset=bass.IndirectOffsetOnAxis(ap=idx_col, axis=0))
                return t

            # ---- gathers for encoder ----
            cen = gather(etab, sfidx_t[:, 0:1], BS, D, "cen", 1)
            cxs = [gather(etab, cidx_t[:, b:b + 1], 128, D, "cx", 3) for b in range(BS)]

            # ---- stage A: encoder ----
            cT = []
            for k in range(2):
                tps = ps.tile([128, BS], f32, tag="psml", bufs=4)
                nc.tensor.transpose(tps[:], cen[:, k * 128:(k + 1) * 128], id_t[0:BS, 0:BS])
                t = sb.tile([128, BS], f32, tag="cT", bufs=2)
                nc.vector.tensor_copy(t[:], tps[:])
                cT.append(t)
            cw_ps = ps.tile([BS, HID], f32, tag="psml", bufs=4)
            nc.tensor.matmul(cw_ps[:], lhsT=cT[0][:], rhs=fWt[0][:], start=True, stop=False)
            nc.tensor.matmul(cw_ps[:], lhsT=cT[1][:], rhs=fWt[1][:], start=False, stop=False)
            nc.tensor.matmul(cw_ps[:], lhsT=on_t[0:1, 0:BS], rhs=fb_t[:, :], start=False, stop=True)
            cwb = sb.tile([BS, HID], f32, tag="cwb")
            nc.vector.tensor_copy(cwb[:], cw_ps[:])
            cwrows = []
            for b in range(BS):
                r = sb.tile([1, HID], f32, tag="cwr", bufs=BS, name=f"cwr{b}")
                nc.sync.dma_start(r[:], cwb[b:b + 1, :])
                cwrows.append(r)

            plT_ps = [ps.tile([128, BS], f32, tag="plt", bufs=2, name=f"plTps{k}") for k in range(2)]
            for b in range(BS):
                cxT = []
                for k in range(2):
                    tps = ps.tile([128, 128], f32, tag="pbig", bufs=2)
                    nc.tensor.transpose(tps[:], cxs[b][:, k * 128:(k + 1) * 128], id_t[:, :])
                    t = sb.tile([128, 128], f32, tag="cxT", bufs=2)
                    nc.vector.tensor_copy(t[:], tps[:])
                    cxT.append(t)
                h_ps = ps.tile([128, HID], f32, tag="pbig", bufs=2)
                nc.tensor.matmul(h_ps[:], lhsT=cxT[0][:], rhs=fWt[2][:], start=True, stop=False)
                nc.tensor.matmul(h_ps[:], lhsT=cxT[1][:], rhs=fWt[3][:], start=False, stop=False)
                nc.tensor.matmul(h_ps[:], lhsT=on_t[0:1, :], rhs=cwrows[b][:], start=False, stop=True)
                hr = sb.tile([128, HID], f32, tag="hr", bufs=2)
                nc.scalar.activation(hr[:], h_ps[:], ACT.Relu)
                for k in range(2):
                    nc.tensor.matmul(plT_ps[k][:, b:b + 1],
                                     lhsT=hr[:, k * 128:(k + 1) * 128],
                                     rhs=on_t[:, 0:1], start=True, stop=True)
            plT = []
            for k in range(2):
                t = sb.tile([128, BS], f32, tag="plT", bufs=2)
                nc.scalar.mul(t[:], plT_ps[k][:], 1.0 / C)
                plT.append(t)
            uv_ps = ps.tile([BS, D + 1], f32, tag="psml", bufs=4)
            nc.tensor.matmul(uv_ps[:], lhsT=plT[0][:], rhs=uvt[0][:], start=True, stop=False)
            nc.tensor.matmul(uv_ps[:], lhsT=plT[1][:], rhs=uvt[1][:], start=False, stop=False)
            nc.tensor.matmul(uv_ps[:], lhsT=on_t[0:1, 0:BS], rhs=uvb_t[:, :], start=False, stop=True)

            psig = sb.tile([BS, 1], f32, tag="psig")
            nc.vector.tensor_copy(psig[:], uv_ps[:, D:D + 1])
            sqq = sb.tile([BS, 1], f32, tag="sqq")
            nc.scalar.activation(sqq[:], psig[:], ACT.Exp, scale=2.0)
            c1 = sb.tile([BS, 1], f32, tag="c1")
            nc.vector.tensor_scalar(c1[:], psig[:], -256.0, -128.0, op0=ALU.mult, op1=ALU.add)
            c2 = sb.tile([BS, 1], f32, tag="c2")
            nc.vector.tensor_scalar(c2[:], sqq[:], 128.0, None, op0=ALU.mult)
            bsrc = sb.tile([BS, D + 3], f32, tag="bsrc")
            nc.vector.tensor_copy(bsrc[:, 0:D], uv_ps[:, 0:D])
            nc.vector.tensor_copy(bsrc[:, D:D + 1], c1[:])
            nc.vector.tensor_copy(bsrc[:, D + 1:D + 2], c2[:])
            nc.vector.tensor_copy(bsrc[:, D + 2:D + 3], gct_t[:])

            nc.sync.dma_start(dbsrc[:, :], bsrc[:])
            score_cols = sb.tile([O, BS], f32, tag="scol")

            # ---- stages B & C per batch element ----
            SR = PR  # 264: gathered row stride inside the per-b big tiles
            nrm_all = sb.tile([O, BS], f32, tag="nrmall")
            nc.vector.tensor_scalar(nrm_all[:], lfct_t[:], 1.0, None, op0=ALU.max)
            rcn_all = sb.tile([O, BS], f32, tag="rcnall")
            nc.vector.reciprocal(rcn_all[:], nrm_all[:])
            qo_all = sb.tile([O, BS], f32, tag="qoall")
            sgn_all = sb.tile([O, BS], f32, tag="sgnall")

            def sview(ap, off, dims):
                return bass.AP(ap.tensor, ap.offset + off, [list(ap.ap[0])] + dims)

            for b in range(BS):
                bc_sb = sb.tile([128, D + 3], f32, tag="bcs", bufs=3)
                nc.sync.dma_start(
                    bc_sb[:], bass.AP(dbsrc[:, :].tensor, b * (D + 3), [[0, 128], [1, D + 3]]))
                gbig = sb.tile([128, 8 * SR], f32, tag="gb", bufs=4)
                for t in range(8):
                    nc.gpsimd.indirect_dma_start(
                        out=gbig[:, t * SR:(t + 1) * SR], out_offset=None, in_=ptab[:, :],
                        in_offset=bass.IndirectOffsetOnAxis(
                            ap=gidx_t[:, 8 * b + t:8 * b + t + 1], axis=0))
                lsgv = sview(gbig[:], D, [[SR, 8]])               # [128, 8] logsig view
                gmuv = sview(gbig[:], 0, [[SR, 8], [1, D]])       # [128, 8, 256]
                bcv = sview(bc_sb[:], 0, [[0, 8], [1, D]])        # bcast along tile dim
                quad = sb.tile([128, 8], f32, tag="quad", bufs=2)
                for h in range(2):
                    gmuh = sview(gbig[:], 4 * h * SR, [[SR, 4], [1, D]])
                    bch = sview(bc_sb[:], 0, [[0, 4], [1, D]])
                    dscb = sb.tile([128, 4, D], f32, tag="dscb", bufs=2, name=f"dscb{h}")
                    nc.vector.tensor_tensor(out=dscb[:], in0=gmuh, in1=bch, op=ALU.subtract)
                    for t in range(4):
                        dsq = sb.tile([128, D], f32, tag="dsq", bufs=1)
                        nc.scalar.activation(dsq[:], dscb[:, t, :], ACT.Square,
                                             accum_out=quad[:, 4 * h + t:4 * h + t + 1])
                # kl on [128, 8]
                einv = sb.tile([128, 8], f32, tag="einv", bufs=2)
                nc.scalar.activation(einv[:], lsgv, ACT.Exp, scale=-2.0)
                t1 = sb.tile([128, 8], f32, tag="t1", bufs=2)
                nc.vector.tensor_scalar(t1[:], quad[:], 0.5, bc_sb[:, D + 1:D + 2],
                                        op0=ALU.mult, op1=ALU.add)
                t2 = sb.tile([128, 8], f32, tag="t2", bufs=2)
                nc.vector.tensor_mul(t2[:], t1[:], einv[:])
                t3 = sb.tile([128, 8], f32, tag="t3", bufs=2)
                nc.vector.tensor_scalar(t3[:], lsgv, 256.0, bc_sb[:, D:D + 1],
                                        op0=ALU.mult, op1=ALU.add)
                klb = sb.tile([128, 8], f32, tag="klb", bufs=2)
                nc.vector.tensor_add(klb[:], t2[:], t3[:])
                cmpm = sb.tile([128, 8], mybir.dt.uint32, tag="cmpm", bufs=2)
                nc.vector.tensor_tensor(
                    out=cmpm[:], in0=gio_t[:], in1=bc_sb[:, D + 2:D + 3].to_broadcast([128, 8]),
                    op=ALU.is_ge)
                nc.vector.copy_predicated(klb[:], cmpm[:], fill_t[:])
                # exact min over 1024
                mn1 = sb.tile([128, 1], f32, tag="mn1", bufs=2)
                nc.vector.tensor_reduce(mn1[:], klb[:], axis=mybir.AxisListType.X, op=ALU.min)
                mn_ps = ps.tile([1, 128], f32, tag="psml", bufs=4)
                nc.tensor.transpose(mn_ps[:], mn1[:], id_t[:, :])
                mns = sb.tile([1, 1], f32, tag="mns", bufs=2)
                nc.vector.tensor_reduce(mns[:], mn_ps[:], axis=mybir.AxisListType.X, op=ALU.min)
                # Match XLA/Eigen exp semantics: gradual subnormals, hard 0 at
                # x <= -97.2865 (measured cutoff, identical on cpu + neuron).
                # ACT's exp spline clamps before f32 underflow, so compute
                # exp(x/2)^2 and apply the cutoff mask explicitly.
                mrow = sb.tile([1, 2], f32, tag="mrow", bufs=2)
                nc.vector.tensor_scalar(mrow[:, 0:1], mns[:], 0.5, None, op0=ALU.mult)
                nc.vector.tensor_scalar(mrow[:, 1:2], mns[:], 1.0, 97.2865, op0=ALU.mult, op1=ALU.add)
                mnb_ps = ps.tile([128, 2], f32, tag="psml", bufs=4)
                nc.tensor.matmul(mnb_ps[:], lhsT=on_t[0:1, :], rhs=mrow[:], start=True, stop=True)
                mnb = sb.tile([128, 2], f32, tag="mnb", bufs=2)
                nc.vector.tensor_copy(mnb[:], mnb_ps[:])
                e_h = sb.tile([128, 8], f32, tag="eh", bufs=2)
                nc.scalar.activation(e_h[:], klb[:], ACT.Exp, scale=-0.5, bias=mnb[:, 0:1])
                e_sq = sb.tile([128, 8], f32, tag="esq", bufs=2)
                nc.vector.tensor_mul(e_sq[:], e_h[:], e_h[:])
                ecut = sb.tile([128, 8], f32, tag="ecut", bufs=2)
                nc.vector.tensor_scalar(ecut[:], klb[:], mnb[:, 1:2], None, op0=ALU.is_lt)
                e_b = sb.tile([128, 8], f32, tag="eb", bufs=2)
                nc.vector.tensor_mul(e_b[:], e_sq[:], ecut[:])
                # weighted posterior accumulate: [sum e*mu | sum e*ls | sum e]
                ws_ps = ps.tile([1, D + 2], f32, tag="psml", bufs=4)
                for t in range(8):
                    nc.tensor.matmul(ws_ps[:], lhsT=e_b[:, t:t + 1],
                                     rhs=gbig[:, t * SR:t * SR + D + 2],
                                     start=(t == 0), stop=(t == 7))
                sigx = sb.tile([128, 8], f32, tag="sigx", bufs=2)
                nc.scalar.activation(sigx[:], lsgv, ACT.Exp)
                scr2 = sb.tile([128, 8], f32, tag="scr2", bufs=2)
                pcol = sb.tile([128, 1], f32, tag="pcol", bufs=2)
                nc.vector.tensor_mul(scr2[:], e_b[:], sigx[:])
                nc.vector.tensor_reduce(pcol[:], scr2[:], axis=mybir.AxisListType.X, op=ALU.add)
                sg_ps = ps.tile([1, 1], f32, tag="psml", bufs=4)
                nc.tensor.matmul(sg_ps[:], lhsT=pcol[:], rhs=on_t[:, 0:1], start=True, stop=True)
                rc = sb.tile([1, 1], f32, tag="rc", bufs=2)
                nc.vector.reciprocal(rc[:], ws_ps[:, D + 1:D + 2])
                # posterior row: mu_post(256) | cc1 | cc2
                post = sb.tile([1, D + 2], f32, tag="post", bufs=2)
                nc.vector.tensor_scalar(post[:, 0:D], ws_ps[:, 0:D], rc[:], None, op0=ALU.mult)
                sgp = sb.tile([1, 1], f32, tag="sgp", bufs=2)
                nc.vector.tensor_scalar(sgp[:], sg_ps[:], rc[:], None, op0=ALU.mult)
                lnsg = sb.tile([1, 1], f32, tag="lnsg", bufs=2)
                nc.scalar.activation(lnsg[:], sgp[:], ACT.Ln)
                nc.vector.tensor_scalar(post[:, D:D + 1], lnsg[:], -256.0, -128.0,
                                        op0=ALU.mult, op1=ALU.add)
                sq2b = sb.tile([1, 1], f32, tag="sq2b", bufs=2)
                nc.vector.tensor_mul(sq2b[:], sgp[:], sgp[:])
                nc.vector.tensor_scalar(post[:, D + 1:D + 2], sq2b[:], 128.0, None, op0=ALU.mult)
                nc.sync.dma_start(ed_all[b, :, :], e_b[:])

                # ---- stage C (per-b part) ----
                nc.sync.dma_start(dpost[b:b + 1, :], post[:])
                bc2_sb = sb.tile([O, D + 2], f32, tag="bc2s", bufs=3)
                nc.sync.dma_start(
                    bc2_sb[:], bass.AP(dpost[:, :].tensor, b * (D + 2), [[0, O], [1, D + 2]]))
                lbig = sb.tile([128, 4 * SR], f32, tag="lb", bufs=2)
                for t in range(4):
                    nc.gpsimd.indirect_dma_start(
                        out=lbig[:, t * SR:(t + 1) * SR], out_offset=None, in_=ptab[:, :],
                        in_offset=bass.IndirectOffsetOnAxis(
                            ap=lfidx_t[:, 4 * b + t:4 * b + t + 1], axis=0))
                nc.scalar.activation(sview(lbig[:], D + 2, [[SR, 4]]),
                                     sview(lbig[:], D, [[SR, 4]]), ACT.Exp)
                lf_ps = ps.tile([O, D + 3], f32, tag="pbig", bufs=2)
                for t in range(4):
                    nc.tensor.matmul(lf_ps[:], lhsT=p4_t[t][:], rhs=lbig[:, t * SR:t * SR + D + 3],
                                     start=(t == 0), stop=(t == 3))
                d1 = sb.tile([O, D + 3], f32, tag="d1", bufs=2)
                nc.vector.tensor_scalar(d1[:], lf_ps[:], rcn_all[:, b:b + 1], None, op0=ALU.mult)
                d2 = sb.tile([O, D], f32, tag="d2", bufs=2)
                nc.vector.tensor_sub(d2[:], d1[:, 0:D], bc2_sb[:, 0:D])
                d2sq = sb.tile([O, D], f32, tag="d2sq", bufs=2)
                nc.scalar.activation(d2sq[:], d2[:], ACT.Square, accum_out=qo_all[:, b:b + 1])
                nc.vector.tensor_copy(sgn_all[:, b:b + 1], d1[:, D + 2:D + 3])

            # ---- stage C batched tail ----
            cc1_all = sb.tile([O, BS], f32, tag="cc1all")
            nc.sync.dma_start(cc1_all[:], bass.AP(dpost[:, :].tensor, D, [[0, O], [D + 2, BS]]))
            cc2_all = sb.tile([O, BS], f32, tag="cc2all")
            nc.sync.dma_start(cc2_all[:], bass.AP(dpost[:, :].tensor, D + 1, [[0, O], [D + 2, BS]]))
            lns_a = sb.tile([O, BS], f32, tag="lnsa")
            nc.scalar.activation(lns_a[:], sgn_all[:], ACT.Ln)
            sq2_a = sb.tile([O, BS], f32, tag="sq2a")
            nc.vector.tensor_mul(sq2_a[:], sgn_all[:], sgn_all[:])
            isq_a = sb.tile([O, BS], f32, tag="isqa")
            nc.vector.reciprocal(isq_a[:], sq2_a[:])
            u1_a = sb.tile([O, BS], f32, tag="u1a")
            nc.vector.tensor_scalar(u1_a[:], qo_all[:], 0.5, None, op0=ALU.mult)
            nc.vector.tensor_add(u1_a[:], u1_a[:], cc2_all[:])
            u2_a = sb.tile([O, BS], f32, tag="u2a")
            nc.vector.tensor_mul(u2_a[:], u1_a[:], isq_a[:])
            u3_a = sb.tile([O, BS], f32, tag="u3a")
            nc.vector.tensor_scalar(u3_a[:], lns_a[:], 256.0, None, op0=ALU.mult)
            nc.vector.tensor_add(u3_a[:], u3_a[:], cc1_all[:])
            klo_a = sb.tile([O, BS], f32, tag="kloa")
            nc.vector.tensor_add(klo_a[:], u2_a[:], u3_a[:])
            nc.vector.tensor_scalar(score_cols[:], klo_a[:], -1.0, None, op0=ALU.mult)

            # ---- batched top-5 indices ----
            erow_all = sb.tile([BS, G], f32, tag="erowall")
            for b in range(BS):
                nc.sync.dma_start(erow_all[b:b + 1, :],
                                  ed_all[b, :, :].rearrange("p t -> t p"))
            t8 = sb.tile([BS, 8], f32, tag="t8")
            nc.vector.max(t8[:], erow_all[:])
            t8i = sb.tile([BS, 8], mybir.dt.uint32, tag="t8i")
            nc.vector.max_index(t8i[:], t8[:], erow_all[:])
            tw = sb.tile([BS, K_TOP], mybir.dt.int32, tag="tw")
            nc.vector.tensor_copy(tw[:], t8i[:, 0:K_TOP])
            nc.sync.dma_start(topw_out[:, :], tw[:])

            # ---- finalize score ----
            sc_ps = ps.tile([BS, O], f32, tag="psml", bufs=4)
            nc.tensor.transpose(sc_ps[:], score_cols[:], id_t[0:O, 0:O])
            scs = sb.tile([BS, O], f32, tag="scs")
            nc.vector.tensor_copy(scs[:], sc_ps[:])
            cmp2 = sb.tile([BS, O], mybir.dt.uint32, tag="cmp2")
            nc.vector.tensor_scalar(cmp2[:], io_t[:, 0:O], nout_t[:], None, op0=ALU.is_ge)
            nc.vector.copy_predicated(scs[:], cmp2[:], ninf_t[:])
            nc.sync.dma_start(score_out[:, :], scs[:])

    if split:
        split_multi_waits(nc, mybir)
    return nc


def host_prep(inputs):
    """Build per-core in_maps (layout/dtype transforms only)."""
    f = lambda x: np.ascontiguousarray(np.asarray(x), dtype=np.float32)
    i = lambda x: np.ascontiguousarray(np.asarray(x), dtype=np.int32)

    ptab = np.zeros((V, PR), np.float32)
    ptab[:, :D] = np.asarray(inputs["emb_mu"], np.float32)
    ptab[:, D] = np.asarray(inputs["emb_log_sigma"], np.float32)[:, 0]
    ptab[:, D + 1] = 1.0
    etab = f(inputs["enc_emb"])
    fW = f(inputs["f_W"])
    fb = f(inputs["f_b"]).reshape(1, HID)
    uvW = np.concatenate([f(inputs["u_W"]), f(inputs["v_W"])], axis=1)
    uvb = np.concatenate([f(inputs["u_b"]), f(inputs["v_b"])]).reshape(1, D + 1)
    ident = np.eye(128, dtype=np.float32)
    onesq = np.ones((128, 128), np.float32)
    pool4 = (np.arange(512)[:, None] // L == np.arange(O)[None, :]).astype(np.float32)
    iota_b = np.tile(np.arange(G, dtype=np.float32), (BS, 1))
    giota = (np.arange(8)[None, :] * 128 + np.arange(128)[:, None]).astype(np.float32)

    sf_ids = i(inputs["sf_ids"])
    context_ids = i(inputs["context_ids"])
    lf_ids = i(inputs["lf_ids"])
    global_ids = i(inputs["global_ids"])
    lf_token_ct = f(inputs["lf_token_ct"])
    gct_all = f(inputs["global_token_ct"]).reshape(B)
    nout_all = f(inputs["num_outputs"]).reshape(B)

    shared = dict(ptab=ptab, etab=etab, fW=fW, fb=fb, uvW=uvW, uvb=uvb,
                  ident=ident, onesq=onesq, pool4=pool4, iota_b=iota_b,
                  giota=giota)
    in_maps = []
    for c in range(NCORES):
        s = slice(c * BS, (c + 1) * BS)
        gi = global_ids[s].reshape(BS, 8, 128).transpose(2, 0, 1).reshape(128, BS * 8)
        li = lf_ids[s].reshape(BS, 512).reshape(BS, 4, 128).transpose(2, 0, 1).reshape(128, BS * 4)
        ci = context_ids[s].T
        in_maps.append(dict(
            shared,
            gidx=np.ascontiguousarray(gi), lfidx=np.ascontiguousarray(li),
            cidx=np.ascontiguousarray(ci),
            sfidx=np.ascontiguousarray(sf_ids[s].reshape(BS, 1)),
            gct=np.ascontiguousarray(gct_all[s].reshape(BS, 1)),
            nout=np.ascontiguousarray(nout_all[s].reshape(BS, 1)),
            lfctT=np.ascontiguousarray(lf_token_ct[s].T),
        ))
    return in_maps


def kernel(**inputs):
    from concourse.bass_utils import run_bass_kernel_spmd

    in_maps = host_prep(inputs)
    if _cache["nc"] is None:
        _cache["nc"] = build_nc()
    res = run_bass_kernel_spmd(_cache["nc"], in_maps, core_ids=list(range(NCORES)))
    _cache["last_res"] = res
    score = np.concatenate([r["score_out"] for r in res.results], axis=0)
    topw = np.concatenate([r["topw_out"] for r in res.results], axis=0)
    return score, np.asarray(inputs["target_lf_ids"]), topw
